# revision 1
# baseline (speedup 1.0000x reference)
"""Mixtral attention layer (B=1, S=2048, H=4096, NH=32, NKV=8, HD=128) on 8
Trainium2 NeuronCores, tensor-parallel over heads.

Sharding: core c owns 4 query heads + 1 KV head (column-shard of wq/wk/wv,
row-shard of wo).  Each core computes a full [S, H] partial of the o_proj
output; the host sums the 8 partials and adds the residual (the gather of a
row-parallel matmul).

Per-core pipeline (projection/attention matmuls in float32r = fp22-truncated
fp32, full PE rate at N>=256):
  Pass A (norm stats): x^T streamed as bf16; ACT squares it, a ones-vector
    matmul reduces sum(x^2) over H (partition reduction on PE) into PSUM;
    r = 1/sqrt(mean+eps) is partition-broadcast (GPSIMD) and folded into
    full-width RoPE cos/sin tables.
  Pass B (projections): x^T re-streamed in fp32r; 6 accumulating matmuls
    per H-chunk produce q^T (4 heads) / k^T / v^T in 6 PSUM banks; the PSUM
    evacuation applies norm + RoPE in 4 tensor ops per tile (DVE for q,
    GPSIMD for k/v).  norm_w is folded into the weights on the host.
  Attention: per head-pair sweep (both heads share this core's single KV
    head - GQA), causal flash-style: scores^T = k^T.T @ q^T chunkwise, exp
    on ACT (PSUM->SBUF), causal mask via GPSIMD affine_select on diagonal
    blocks, unnormalized AV + ones-matmul row-sum Z accumulate in PSUM; 1/Z
    applied at AV evacuation into SBUF-resident attn^T.
  o_proj: attn^T @ wo accumulated over the 4 heads, DMA'd out per tile.

q^T is spilled to internal DRAM between phases (SBUF pressure); attn^T
reuses the wk/wv SBUF slots after the projections retire.
"""

import math

import numpy as np

import concourse.bass as bass
import concourse.tile as tile
from concourse import bacc, mybir
from concourse.masks import make_identity

F32 = mybir.dt.float32
F32R = mybir.dt.float32r
BF16 = mybir.dt.bfloat16

# Full problem dims
B, S, H, NH, NKV, HD = 1, 2048, 4096, 32, 8, 128
EPS = 1e-5
N_CORES = 8
QH = NH // N_CORES          # query heads per core = 4
DQ = QH * HD                # q columns per core = 512
DKV = (NKV // N_CORES) * HD  # kv columns per core = 128


def build_bass(s=S, h=H, qh=QH, stop_after=None, diag=None):
    """Build the single-core Bass module (same NEFF on all 8 cores)."""
    ST = 512 if s >= 512 else s       # s-tile width (proj + attention i-tiles)
    NST = s // ST                     # number of s-tiles
    HC = h // 128                     # H contraction chunks
    NJ = s // 128                     # j chunks (keys)
    NSC = s // 128                    # s chunks for o_proj
    NHT = h // 512 if h >= 512 else 1  # h tiles for o_proj output
    HT = min(512, h)
    dq = qh * HD
    scale = 1.0 / math.sqrt(HD)

    nc = bacc.Bacc(None, target_bir_lowering=False)

    xT = nc.dram_tensor("xT", [h, s], F32R, kind="ExternalInput")
    xTb = nc.dram_tensor("xTb", [h, s], BF16, kind="ExternalInput")
    wq = nc.dram_tensor("wq", [h, dq], F32R, kind="ExternalInput")
    wk = nc.dram_tensor("wk", [h, DKV], F32R, kind="ExternalInput")
    wv = nc.dram_tensor("wv", [h, DKV], F32R, kind="ExternalInput")
    wo = nc.dram_tensor("wo", [dq, h], F32R, kind="ExternalInput")
    cosT = nc.dram_tensor("cosT", [HD, s], F32, kind="ExternalInput")
    sinTs = nc.dram_tensor("sinTs", [HD, s], F32, kind="ExternalInput")
    out = nc.dram_tensor("out", [s, h], F32, kind="ExternalOutput")

    xT_t = xT.rearrange("(ho hi) s -> hi ho s", hi=128)
    xTb_t = xTb.rearrange("(ho hi) s -> hi ho s", hi=128)
    wq_t = wq.rearrange("(ho hi) d -> hi ho d", hi=128)
    wk_t = wk.rearrange("(ho hi) d -> hi ho d", hi=128)
    wv_t = wv.rearrange("(ho hi) d -> hi ho d", hi=128)
    wo_t = wo.rearrange("(do di) h -> di do h", di=128)

    with tile.TileContext(nc) as tc:
        with (
            tc.tile_pool(name="persist", bufs=1) as persist,
            tc.tile_pool(name="xin", bufs=4) as xin,
            tc.tile_pool(name="xbin", bufs=3) as xbin,
            tc.tile_pool(name="x2b", bufs=3) as x2b,
            tc.tile_pool(name="rope", bufs=3) as ropep,
            tc.tile_pool(name="statp", bufs=4) as statp,
            tc.tile_pool(name="tabp", bufs=2) as tabp,
            tc.tile_pool(name="bcastp", bufs=3) as bcastp,
            tc.tile_pool(name="probs", bufs=6) as probs,
            tc.tile_pool(name="outp", bufs=3) as outp,
            tc.tile_pool(name="qin", bufs=3) as qin,
            tc.tile_pool(name="dramp", bufs=1, space="DRAM") as dramp,
            tc.tile_pool(name="acc_ps", bufs=8, space="PSUM") as acc_ps,
        ):
            # ---- persistent SBUF tensors ----
            # Slot reuse chains (same tag, sequential lifetimes):
            #   wq (8MB) -> wo (8MB)         tag "bigw"
            #   wk (2MB) -> attnT heads 0-1  tag "wk"
            #   wv (2MB) -> attnT heads 2-3  tag "wv"
            #   cos (1MB) -> v natural (1MB) tag "cosvnat"
            wq_sb = persist.tile([128, HC, dq], F32R, tag="bigw")
            wk_sb = persist.tile([128, HC, DKV], F32R, tag="wk")
            wv_sb = persist.tile([128, HC, DKV], F32R, tag="wv")
            cos_sb = persist.tile([128, s], F32, tag="cosvnat")
            sin_sb = persist.tile([128, s], F32, tag="sin")
            ones_f = persist.tile([128, 1], F32, tag="ones_f")
            ones_sb = persist.tile([128, 1], F32R, tag="ones")
            ones_bf = persist.tile([128, 1], BF16, tag="ones_bf")
            eps_sb = persist.tile([1, 1], F32, tag="eps")
            ident_sb = persist.tile([128, 128], F32, tag="ident")
            kT_sb = persist.tile([128, s], F32R, tag="kT")
            vT_sb = persist.tile([128, s], F32, tag="vT")
            # q^T spilled to DRAM, re-streamed by attention
            qT_dr = dramp.tile([128, qh, s], F32R, tag="qT_dr")

            nc.sync.dma_start(out=wq_sb, in_=wq_t)
            nc.sync.dma_start(out=wk_sb, in_=wk_t)
            nc.sync.dma_start(out=wv_sb, in_=wv_t)
            nc.sync.dma_start(out=cos_sb, in_=cosT[:, :])
            nc.sync.dma_start(out=sin_sb, in_=sinTs[:, :])
            nc.vector.memset(ones_f, 1.0)
            nc.scalar.copy(ones_sb, ones_f)
            nc.scalar.copy(ones_bf, ones_f)
            nc.vector.memset(eps_sb, EPS)
            make_identity(nc, ident_sb)

            # ---- phase 1: interleaved pass A (norm stats, bf16) and
            # pass B (q/k/v projections, fp32r), pass A one s-tile ahead ----
            def pass_a(st):
                ss = bass.ts(st, ST)
                sq_ps = acc_ps.tile([1, ST], F32, tag="acc", name="sq_ps")
                for hc in range(HC):
                    xb_sb = xbin.tile([128, ST], BF16)
                    nc.sync.dma_start(out=xb_sb, in_=xTb_t[:, hc, ss])
                    x2_sb = x2b.tile([128, ST], BF16)
                    nc.scalar.square(x2_sb, xb_sb)
                    nc.tensor.matmul(sq_ps, ones_bf, x2_sb,
                                     start=(hc == 0), stop=(hc == HC - 1))
                # r = 1/sqrt(mean + eps); fold into cos/sin tables
                sd_sb = statp.tile([1, ST], F32, tag="stat", name="sd_sb")
                nc.scalar.activation(
                    sd_sb, sq_ps, mybir.ActivationFunctionType.Sqrt,
                    bias=eps_sb, scale=1.0 / h,
                )
                rr_sb = statp.tile([1, ST], F32, tag="stat", name="rr_sb")
                nc.vector.reciprocal(rr_sb, sd_sb)
                R_t = tabp.tile([128, ST], F32, tag="R", name="R_t")
                nc.gpsimd.partition_broadcast(R_t, rr_sb)
                cp_t = tabp.tile([128, ST], F32, tag="cp", name="cp_t")
                nc.vector.tensor_mul(cp_t, cos_sb[:, ss], R_t)
                sp_t = tabp.tile([128, ST], F32, tag="sp", name="sp_t")
                nc.vector.tensor_mul(sp_t, sin_sb[:, ss], R_t)
                return R_t, cp_t, sp_t

            def pass_b(st, tabs):
                R_t, cp_t, sp_t = tabs
                ss = bass.ts(st, ST)
                q_ps = [acc_ps.tile([128, ST], F32, tag="acc", name=f"q_ps{m}")
                        for m in range(qh)]
                k_ps = acc_ps.tile([128, ST], F32, tag="acc", name="k_ps")
                v_ps = acc_ps.tile([128, ST], F32, tag="acc", name="v_ps")
                for hc in range(HC):
                    x_sb = xin.tile([128, ST], F32R)
                    nc.sync.dma_start(out=x_sb, in_=xT_t[:, hc, ss])
                    st_, sp_ = (hc == 0), (hc == HC - 1)
                    for m in range(qh):
                        nc.tensor.matmul(
                            q_ps[m], wq_sb[:, hc, bass.ts(m, 128)], x_sb,
                            start=st_, stop=sp_,
                        )
                    nc.tensor.matmul(k_ps, wk_sb[:, hc, :], x_sb,
                                     start=st_, stop=sp_)
                    nc.tensor.matmul(v_ps, wv_sb[:, hc, :], x_sb,
                                     start=st_, stop=sp_)
                # evacuation: fast ACT copy frees the PSUM bank, then
                # norm+RoPE happens SBUF-side on DVE (in place; the u-halves
                # read the raw values before the cos-multiply overwrites)
                def rope_evac(src_ps, dst):
                    u_sb = ropep.tile([128, ST], F32, tag="u", name="u_sb",
                                      bufs=2)
                    nc.scalar.copy(dst, src_ps)
                    nc.vector.tensor_mul(
                        u_sb[0:64, :], dst[64:128, :], sp_t[64:128, :])
                    nc.vector.tensor_mul(
                        u_sb[64:128, :], dst[0:64, :], sp_t[0:64, :])
                    nc.vector.tensor_mul(dst, dst, cp_t)
                    nc.vector.tensor_add(dst, dst, u_sb)

                for m in range(qh if diag != "no_evac" else 0):
                    dst = ropep.tile([128, ST], F32R, tag="t", name="t_sb",
                                     bufs=4)
                    rope_evac(q_ps[m], dst)
                    nc.sync.dma_start(out=qT_dr[:, m, ss], in_=dst)
                if diag == "no_evac":
                    return
                rope_evac(k_ps, kT_sb[:, ss])
                nc.scalar.copy(vT_sb[:, ss], v_ps)
                nc.vector.tensor_mul(vT_sb[:, ss], vT_sb[:, ss], R_t)

            if diag == "no_pa":
                R_t = tabp.tile([128, ST], F32, tag="R", name="R_t")
                cp_t = tabp.tile([128, ST], F32, tag="cp", name="cp_t")
                sp_t = tabp.tile([128, ST], F32, tag="sp", name="sp_t")
                nc.vector.memset(R_t, 1.0)
                nc.vector.memset(cp_t, 1.0)
                nc.vector.memset(sp_t, 1.0)
                for st in range(NST):
                    pass_b(st, (R_t, cp_t, sp_t))
            else:
                tabs = pass_a(0)
                for st in range(NST):
                    pass_b(st, tabs)
                    if st + 1 < NST:
                        tabs = pass_a(st + 1)

            # ---- phase 2: transpose v to natural [j, d] layout ----
            vnat_sb = persist.tile([128, NJ, 128], F32R, tag="cosvnat")
            wo_sb = persist.tile([128, qh, h], F32R, tag="bigw")
            if stop_after != "p1":
                nc.sync.dma_start(out=wo_sb, in_=wo_t)
            for jc in range(NJ if stop_after != "p1" else 0):
                vt_ps = acc_ps.tile([128, 128], F32, tag="acc")
                nc.tensor.transpose(vt_ps, vT_sb[:, bass.ts(jc, 128)], ident_sb)
                nc.scalar.copy(vnat_sb[:, jc, :], vt_ps)

            # attn^T reuses the wk/wv slots (heads 0-1 / 2-3)
            attnT_h = [
                persist.tile([128, 2, s], F32R, tag="wk", name="attnT01"),
                persist.tile([128, 2, s], F32R, tag="wv", name="attnT23"),
            ]

            def attn_slice(m, sl):
                return attnT_h[m // 2][:, m % 2, sl]

            # ---- phase 3 + 4 interleaved: attention per i-tile (both
            # head pairs), then immediately the o_proj matmuls for that
            # i-range so they fill PE stalls in the next i-tile's attention
            def attn_tile(hp, ti):
                heads = (2 * hp, 2 * hp + 1)
                if True:
                    iss = bass.ts(ti, ST)
                    q_sbs = []
                    for hh in heads:
                        q_sb = qin.tile([128, ST], F32R, tag="q",
                                        name=f"q_sb{hh}")
                        nc.sync.dma_start(out=q_sb, in_=qT_dr[:, hh, iss])
                        q_sbs.append(q_sb)
                    av_ps = [acc_ps.tile([128, ST], F32, tag="acc",
                                         name=f"av_ps{i}") for i in range(2)]
                    z_ps = [acc_ps.tile([1, ST], F32, tag="acc",
                                        name=f"z_ps{i}") for i in range(2)]
                    njc = (ti + 1) * (ST // 128)
                    for jc in range(njc):
                        st_, sp_ = (jc == 0), (jc == njc - 1)
                        diag = (jc + 1) * 128 > ti * ST
                        for i in range(2):
                            s_ps = acc_ps.tile([128, ST], F32, tag="acc",
                                               name=f"s_ps{i}")
                            nc.tensor.matmul(
                                s_ps, kT_sb[:, bass.ts(jc, 128)], q_sbs[i],
                                start=True, stop=True,
                            )
                            p_sb = probs.tile([128, ST], F32R, tag="p",
                                              name=f"p_sb{i}", bufs=6)
                            nc.scalar.activation(
                                p_sb, s_ps, mybir.ActivationFunctionType.Exp,
                                scale=scale,
                            )
                            if diag:
                                nc.gpsimd.affine_select(
                                    out=p_sb, in_=p_sb,
                                    pattern=[[1, ST]],
                                    compare_op=mybir.AluOpType.is_ge,
                                    fill=0.0,
                                    base=ti * ST - jc * 128,
                                    channel_multiplier=-1,
                                )
                            nc.tensor.matmul(av_ps[i], vnat_sb[:, jc, :], p_sb,
                                             start=st_, stop=sp_)
                            nc.tensor.matmul(z_ps[i], ones_sb, p_sb,
                                             start=st_, stop=sp_)
                    for i, hh in enumerate(heads):
                        zr_sb = statp.tile([1, ST], F32, tag="stat",
                                           name="zr_sb")
                        nc.vector.reciprocal(zr_sb, z_ps[i])
                        ZR_sb = bcastp.tile([128, ST], F32, tag="bcast",
                                            name="ZR_sb")
                        nc.gpsimd.partition_broadcast(ZR_sb, zr_sb)
                        nc.vector.tensor_mul(attn_slice(hh, iss), av_ps[i],
                                             ZR_sb)

            def o_proj_chunk(sc):
                scs = bass.ts(sc, 128)
                for ht in range(NHT):
                    o_ps = acc_ps.tile([128, HT], F32, tag="acc")
                    for m in range(qh):
                        nc.tensor.matmul(
                            o_ps, attn_slice(m, scs),
                            wo_sb[:, m, bass.ts(ht, HT)],
                            start=(m == 0), stop=(m == qh - 1),
                        )
                    o_sb = outp.tile([128, HT], F32)
                    if (sc + ht) % 2 == 0:
                        nc.scalar.copy(o_sb, o_ps)
                    else:
                        nc.vector.tensor_copy(o_sb, o_ps)
                    dma_eng = nc.sync
                    dma_eng.dma_start(
                        out=out[scs, bass.ts(ht, HT)], in_=o_sb
                    )

            if stop_after not in ("p1", "p2"):
                for ti in range(NST):
                    for hp in range(qh // 2):
                        attn_tile(hp, ti)
                    if stop_after is None:
                        for sc in range(ti * (ST // 128), (ti + 1) * (ST // 128)):
                            o_proj_chunk(sc)

    nc.compile()
    return nc


def make_core_inputs(hidden_states, cos, sin, norm_w, wq, wk, wv, wo,
                     s=S, h=H, qh=QH, n_cores=N_CORES):
    """Host-side sharding + layout preparation. Returns list of in_maps."""
    import ml_dtypes

    dq = qh * HD
    dkv = DKV
    x = np.asarray(hidden_states, dtype=np.float32).reshape(s, h)
    nw = np.asarray(norm_w, dtype=np.float32)
    xT = np.ascontiguousarray(x.T)                      # [h, s]
    xTb = np.ascontiguousarray(xT.astype(ml_dtypes.bfloat16))
    cosT = np.ascontiguousarray(np.asarray(cos, np.float32).reshape(s, HD).T)
    sinT = np.ascontiguousarray(np.asarray(sin, np.float32).reshape(s, HD).T)
    # swapped/sign-flipped sin table: rows 0:64 = +sin_half, 64:128 = -sin_half
    sin_half = sinT[0:64]
    sinTs = np.ascontiguousarray(np.concatenate([sinT[64:128], -sin_half], axis=0))
    # fold norm_w into the projection weights
    wq_f = np.asarray(wq, np.float32) * nw[:, None]
    wk_f = np.asarray(wk, np.float32) * nw[:, None]
    wv_f = np.asarray(wv, np.float32) * nw[:, None]
    wo_f = np.asarray(wo, np.float32)

    in_maps = []
    for c in range(n_cores):
        in_maps.append({
            "xT": xT,
            "xTb": xTb,
            "wq": np.ascontiguousarray(wq_f[:, c * dq:(c + 1) * dq]),
            "wk": np.ascontiguousarray(wk_f[:, c * dkv:(c + 1) * dkv]),
            "wv": np.ascontiguousarray(wv_f[:, c * dkv:(c + 1) * dkv]),
            "wo": np.ascontiguousarray(wo_f[c * dq:(c + 1) * dq, :]),
            "cosT": cosT,
            "sinTs": sinTs,
        })
    return in_maps


_NC_CACHE = {}


def kernel(hidden_states, cos, sin, norm_w, wq, wk, wv, wo):
    from concourse.bass_utils import run_bass_kernel_spmd

    if "nc" not in _NC_CACHE:
        _NC_CACHE["nc"] = build_bass()
    nc = _NC_CACHE["nc"]
    in_maps = make_core_inputs(hidden_states, cos, sin, norm_w, wq, wk, wv, wo)
    res = run_bass_kernel_spmd(nc, in_maps, core_ids=list(range(N_CORES)))
    partials = [m["out"] for m in res.results]
    out = np.asarray(hidden_states, np.float32).reshape(S, H).copy()
    for p in partials:
        out += p
    return out.reshape(B, S, H)



# revision 69
# speedup vs baseline: 1.4355x; 1.4355x over previous
"""Mixtral attention layer (B=1, S=2048, H=4096, NH=32, NKV=8, HD=128) on 8
Trainium2 NeuronCores, tensor-parallel over heads.

Sharding: core c owns 4 query heads + 1 KV head (column-shard of wq/wk/wv,
row-shard of wo).  Each core computes a full [S, H] partial of the o_proj
output (bf16); the host sums the 8 partials and adds the residual (the
gather of a row-parallel matmul).

Key ideas (PE matmul cost is output-free-size x cycles/row; fp32r at
free>=256 and bf16 run at 1 cycle/row):
  * Partition-dim reductions (RMSNorm sum(x^2), softmax row-sums Z) use
    FLIPPED matmuls: the data tile is the stationary operand and a ones
    vector the moving one, so each costs ~1 output column instead of the
    tile width.  Each flipped matmul is single-shot (start+stop) into a
    per-chunk-pair region of a scratch PSUM bank (concurrently-open
    accumulation groups in one PSUM bank are illegal: a start wipes the
    other open groups); a DVE add folds pairs into an SBUF accumulator.
    The [s-on-partitions] results are rotated back to [1,s] rows with a
    PE transpose (inputs spread to columns 0/32/64/96 so the transposed
    rows land on 32-aligned partition bases, which DVE reads require).
  * x stream and all weights are bf16 (walrus forbids mixing f32r with
    bf16 in one matmul, so pairs are bf16xbf16); the attention
    internals (kT/qT/vnat/probs, all accumulated in fp32 PSUM) stay
    f32r; attnT/wo (o_proj pair) and the output partials are bf16.
  * Phase 1 fuses norm stats into the projection stream (x read once):
    per H-chunk, 6 accumulating projection matmuls + ACT square + 4
    flipped stats matmuls (lagged 4 chunks so tile starts are pure
    projection work).  Weight DMAs are chunked and interleaved with
    tile-0 x chunks so the PE starts ~4us in; wo streams during tiles
    1-3 into its own SBUF slot; v is transposed to natural [j, d]
    layout mid-next-tile.  norm_w folds into the weights on the host,
    the RMSNorm 1/rms into the RoPE cos/sin tables; q^T/k^T go straight
    to persistent SBUF (no DRAM spill).
  * Attention (causal flash-style, per head-pair per 512-wide i-tile):
    scores^T = kT.T @ qT chunkwise, exp on ACT, causal mask via GPSIMD
    affine_select on diagonal blocks, unnormalized AV accumulating in
    PSUM, Z via flipped matmuls; 1/Z applied at AV evacuation.  The
    previous i-tile's o_proj pieces are emitted one at a time between
    j-chunks so the scheduler can fill the exp->mask->AV latency chains
    with o_proj matmuls.
"""

import math

import numpy as np

import concourse.bass as bass
import concourse.tile as tile
from concourse import bacc, mybir
from concourse.masks import make_identity

F32 = mybir.dt.float32
F32R = mybir.dt.float32r
BF16 = mybir.dt.bfloat16

# Full problem dims
B, S, H, NH, NKV, HD = 1, 2048, 4096, 32, 8, 128
EPS = 1e-5
N_CORES = 8
QH = NH // N_CORES          # query heads per core = 4
DQ = QH * HD                # q columns per core = 512
DKV = (NKV // N_CORES) * HD  # kv columns per core = 128


def build_bass(s=S, h=H, qh=QH, stop_after=None):
    """Build the single-core Bass module (same NEFF on all 8 cores)."""
    ST = 512 if s >= 512 else s       # s-tile width (proj + attention i-tiles)
    NST = s // ST                     # number of s-tiles
    HC = h // 128                     # H contraction chunks
    NJ = s // 128                     # j chunks (keys)
    NHT = h // 512 if h >= 512 else 1  # h tiles for o_proj output
    HT = min(512, h)
    WGRP = 4                          # h-chunks per weight DMA group
    dq = qh * HD
    scale = 1.0 / math.sqrt(HD)

    nc = bacc.Bacc(None, target_bir_lowering=False)

    xT = nc.dram_tensor("xT", [h, s], BF16, kind="ExternalInput")
    wq = nc.dram_tensor("wq", [h, dq], BF16, kind="ExternalInput")
    wk = nc.dram_tensor("wk", [h, DKV], BF16, kind="ExternalInput")
    wv = nc.dram_tensor("wv", [h, DKV], BF16, kind="ExternalInput")
    wo = nc.dram_tensor("wo", [dq, h], BF16, kind="ExternalInput")
    cosT = nc.dram_tensor("cosT", [HD, s], BF16, kind="ExternalInput")
    sinTs = nc.dram_tensor("sinTs", [HD, s], BF16, kind="ExternalInput")
    out = nc.dram_tensor("out", [s, h], BF16, kind="ExternalOutput")

    xT_t = xT.rearrange("(ho hi) s -> hi ho s", hi=128)
    wq_t = wq.rearrange("(ho hi) d -> hi ho d", hi=128)
    wk_t = wk.rearrange("(ho hi) d -> hi ho d", hi=128)
    wv_t = wv.rearrange("(ho hi) d -> hi ho d", hi=128)
    wo_t = wo.rearrange("(do di) h -> di do h", di=128)

    with tile.TileContext(nc) as tc:
        with (
            tc.tile_pool(name="persist", bufs=1) as persist,
            tc.tile_pool(name="xin", bufs=4) as xin,
            tc.tile_pool(name="x2b", bufs=5) as x2b,
            tc.tile_pool(name="rope", bufs=3) as ropep,
            tc.tile_pool(name="statp", bufs=6) as statp,
            tc.tile_pool(name="tabp", bufs=3) as tabp,
            tc.tile_pool(name="bcastp", bufs=3) as bcastp,
            tc.tile_pool(name="probs", bufs=6) as probs,
            tc.tile_pool(name="outp", bufs=6) as outp,
            tc.tile_pool(name="acc_ps", bufs=8, space="PSUM") as acc_ps,
        ):
            # ---- persistent SBUF tensors ----
            # Slot reuse chains (same tag, sequential lifetimes):
            #   wq (8MB) -> wo (8MB)         tag "bigw"
            #   wk (2MB) -> attnT heads 0-1  tag "wk"
            #   wv (2MB) -> attnT heads 2-3  tag "wv"
            #   cos (1MB) -> v natural (1MB) tag "cosvnat"
            wq_sb = persist.tile([128, HC, dq], BF16, tag="bigw")
            wk_sb = persist.tile([128, HC, DKV], BF16, tag="wk")
            wv_sb = persist.tile([128, HC, DKV], BF16, tag="wv")
            cos_sb = persist.tile([128, s], BF16, tag="cos")
            sin_sb = persist.tile([128, s], BF16, tag="sin")
            ones_f = persist.tile([128, 1], F32, tag="ones_f")
            ones_sb = persist.tile([128, 2], F32R, tag="ones")
            ones_bf = persist.tile([128, 1], BF16, tag="ones_bf")
            eps_sb = persist.tile([128, 1], F32, tag="eps")
            ident_sb = persist.tile([128, 128], F32, tag="ident")
            kT_sb = persist.tile([128, s], BF16, tag="kT")
            vT_sb = persist.tile([128, s], F32, tag="vT")
            qT_sb = persist.tile([128, qh, s], BF16, tag="qT")
            wo_sb = persist.tile([128, qh, h], BF16, tag="wo")
            vnat_sb = persist.tile([128, NJ, 128], BF16, tag="vnat")

            nc.vector.memset(ones_f, 1.0)
            nc.scalar.copy(ones_sb[:, 0:1], ones_f)
            nc.scalar.copy(ones_sb[:, 1:2], ones_f)
            nc.scalar.copy(ones_bf, ones_f)
            nc.vector.memset(eps_sb, EPS)
            make_identity(nc, ident_sb)

            # ---- phase 1: fused norm stats + q/k/v projections off a single
            # fp32r x^T stream; weight DMAs interleaved with tile-0 x chunks
            def pass_b(st):
                ss = bass.ts(st, ST)
                q_ps = [acc_ps.tile([128, ST], F32, tag="acc", name=f"q_ps{m}")
                        for m in range(qh)]
                k_ps = acc_ps.tile([128, ST], F32, tag="acc", name="k_ps")
                v_ps = acc_ps.tile([128, ST], F32, tag="acc", name="v_ps")
                # sum(x^2) accumulates in SBUF: each chunk's flipped
                # matmuls are single-shot (start+stop) into a transient PSUM
                # tile -- concurrently-open accumulation groups in one PSUM
                # bank are illegal (a start wipes the other open groups)
                # one scratch PSUM bank per s-tile: cols 0:8 hold the
                # per-chunk-pair stats groups, 16:144 the r transpose
                scr = acc_ps.tile([128, 144], F32, tag="acc", name="p1scr")
                sq_acc = statp.tile([128, 8], F32, tag="sqacc",
                                    name="sq_acc")
                nc.vector.memset(sq_acc, 0.0)
                # weight DMA groups for tile 0: small groups first so the
                # PE can start early, then 4-chunk groups
                wgroups = [(0, 1), (1, 1), (2, 2)] + [
                    (g, WGRP) for g in range(4, HC, WGRP)]

                def stats(hc):
                    # flipped stats: sum over h of x^2 lands on the s
                    # partitions; costs ~1 output column per matmul.  Each
                    # matmul is single-shot (concurrently-open accumulation
                    # groups in one PSUM bank are illegal); a chunk pair
                    # lands in scratch cols 0:8, then one DVE add folds it
                    # into the SBUF accumulator.
                    off = 4 * (hc % 2)
                    for c in range(4):
                        nc.tensor.matmul(
                            scr[:, off + c:off + c + 1],
                            x2s[hc][:, bass.ts(c, 128)],
                            ones_bf, start=True, stop=True,
                        )
                    if hc % 2 == 1:
                        nc.vector.tensor_add(sq_acc, sq_acc, scr[:, 0:8])
                    x2s[hc] = None

                x2s = {}
                for hc in range(HC):
                    x_sb = xin.tile([128, ST], BF16)
                    nc.sync.dma_start(out=x_sb, in_=xT_t[:, hc, ss])
                    if st == 0 and wgroups and wgroups[0][0] == hc:
                        g0, gn = wgroups.pop(0)
                        nc.sync.dma_start(out=wq_sb[:, g0:g0 + gn, :],
                                          in_=wq_t[:, g0:g0 + gn, :])
                        nc.sync.dma_start(out=wk_sb[:, g0:g0 + gn, :],
                                          in_=wk_t[:, g0:g0 + gn, :])
                        nc.sync.dma_start(out=wv_sb[:, g0:g0 + gn, :],
                                          in_=wv_t[:, g0:g0 + gn, :])
                    if st > 0 and hc in (6, 14, 22):
                        # o_proj weights trickle in during tiles 1-3
                        ht = 3 * (st - 1) + (hc - 6) // 8
                        if ht < NHT:
                            nc.sync.dma_start(
                                out=wo_sb[:, :, bass.ts(ht, HT)],
                                in_=wo_t[:, :, bass.ts(ht, HT)],
                            )
                    x2_sb = x2b.tile([128, ST], BF16)
                    nc.scalar.square(x2_sb, x_sb)
                    x2s[hc] = x2_sb
                    st_, sp_ = (hc == 0), (hc == HC - 1)
                    for m in range(qh):
                        nc.tensor.matmul(
                            q_ps[m], wq_sb[:, hc, bass.ts(m, 128)], x_sb,
                            start=st_, stop=sp_,
                        )
                    nc.tensor.matmul(k_ps, wk_sb[:, hc, :], x_sb,
                                     start=st_, stop=sp_)
                    nc.tensor.matmul(v_ps, wv_sb[:, hc, :], x_sb,
                                     start=st_, stop=sp_)
                    # stats lag the stream so tile starts are pure
                    # projection work
                    if hc >= 4:
                        stats(hc - 4)
                    # previous s-tile's v -> natural [j, d] transposes,
                    # placed mid-tile where PSUM banks have slack
                    if st > 0 and hc in (16, 18, 20, 22):
                        jc = (st - 1) * (ST // 128) + (hc - 16) // 2
                        vt_ps = acc_ps.tile([128, 128], F32, tag="acc")
                        nc.tensor.transpose(
                            vt_ps, vT_sb[:, bass.ts(jc, 128)], ident_sb)
                        nc.scalar.copy(vnat_sb[:, jc, :], vt_ps)
                if st == 0:
                    # rope tables: after tile-0's weights, before the first
                    # evacuation needs them
                    nc.sync.dma_start(out=cos_sb, in_=cosT[:, :])
                    nc.sync.dma_start(out=sin_sb, in_=sinTs[:, :])
                for hcl in range(HC - 4, HC):
                    stats(hcl)
                # r = 1/sqrt(mean + eps) in [s-part, 4] layout, rotated back
                # to a [1, ST] row for the table broadcast
                sq_f = statp.tile([128, 4], F32, tag="stat4f",
                                  name="sq_f")
                nc.vector.tensor_add(sq_f, sq_acc[:, 0:4], sq_acc[:, 4:8])
                sd_sb = statp.tile([128, 4], F32, tag="stat4",
                                   name="sd_sb")
                nc.scalar.activation(
                    sd_sb, sq_f, mybir.ActivationFunctionType.Sqrt,
                    bias=eps_sb, scale=1.0 / h,
                )
                # reciprocals spread to columns 0/32/64/96 so the
                # transpose lands them on 32-aligned partitions (DVE reads
                # require 32-aligned partition bases)
                rr_sb = statp.tile([128, 4, 32], F32, tag="stat4b",
                                   name="rr_sb")
                for c in range(4):
                    nc.vector.reciprocal(rr_sb[:, c, 0:1], sd_sb[:, c:c + 1])
                rT_ps = scr[:, 16:144]
                nc.tensor.transpose(rT_ps, rr_sb, ident_sb)
                rf_sb = statp.tile([1, ST], BF16, tag="statfb",
                                   name="rf_sb", bufs=3)
                for c in range(4):
                    nc.vector.tensor_copy(
                        rf_sb[0:1, bass.ts(c, 128)],
                        rT_ps[32 * c:32 * c + 1, :])
                R_t = tabp.tile([128, ST], BF16, tag="R", name="R_t")
                nc.gpsimd.partition_broadcast(R_t, rf_sb)
                cp_t = tabp.tile([128, ST], BF16, tag="cp", name="cp_t")
                nc.vector.tensor_mul(cp_t, cos_sb[:, ss], R_t)
                sp_t = tabp.tile([128, ST], BF16, tag="sp", name="sp_t")
                nc.vector.tensor_mul(sp_t, sin_sb[:, ss], R_t)

                # evacuation: fast ACT copy frees the PSUM bank, then
                # norm+RoPE happens SBUF-side on DVE (in place; the u-halves
                # read the raw values before the cos-multiply overwrites)
                # all PSUM->SBUF copies first, alternating ACT/DVE, so
                # the banks free ~2x faster for the next tile; norm+RoPE
                # then happens SBUF-side on DVE (in place; the u-halves
                # read the raw values before the cos-multiply overwrites)
                evacs = [(k_ps, kT_sb[:, ss])]
                evacs += [(q_ps[m], qT_sb[:, m, ss]) for m in range(qh)]
                for idx, (src_ps, dst) in enumerate(evacs):
                    if idx % 2 == 0:
                        nc.scalar.copy(dst, src_ps)
                    else:
                        nc.vector.tensor_copy(dst, src_ps)
                nc.scalar.copy(vT_sb[:, ss], v_ps)

                def rope_rot(dst):
                    u_sb = ropep.tile([128, ST], BF16, tag="u",
                                      name="u_sb", bufs=3)
                    nc.vector.tensor_mul(
                        u_sb[0:64, :], dst[64:128, :], sp_t[64:128, :])
                    nc.vector.tensor_mul(
                        u_sb[64:128, :], dst[0:64, :], sp_t[0:64, :])
                    nc.vector.tensor_mul(dst, dst, cp_t)
                    nc.vector.tensor_add(dst, dst, u_sb)

                for _, dst in evacs:
                    rope_rot(dst)
                nc.vector.tensor_mul(vT_sb[:, ss], vT_sb[:, ss], R_t)

            for st in range(NST):
                pass_b(st)

            # ---- phase 2: last s-tile's v transposes + the last wo
            # chunks (the rest streamed during tiles 1-3) ----
            if stop_after != "p1":
                for ht in range(min(3 * (NST - 1), NHT), NHT):
                    nc.sync.dma_start(
                        out=wo_sb[:, :, bass.ts(ht, HT)],
                        in_=wo_t[:, :, bass.ts(ht, HT)],
                    )
            for jc in range((NST - 1) * (ST // 128),
                            NJ if stop_after != "p1" else 0):
                vt_ps = acc_ps.tile([128, 128], F32, tag="acc")
                nc.tensor.transpose(vt_ps, vT_sb[:, bass.ts(jc, 128)],
                                    ident_sb)
                nc.scalar.copy(vnat_sb[:, jc, :], vt_ps)

            # attn^T reuses the wk/wv slots (heads 0-1 / 2-3)
            attnT_h = [
                persist.tile([128, 2, s], BF16, tag="wk", name="attnT01"),
                persist.tile([128, 2, s], BF16, tag="wv", name="attnT23"),
            ]

            def attn_slice(m, sl):
                return attnT_h[m // 2][:, m % 2, sl]

            # ---- phase 3 + 4 interleaved: attention per i-tile (both head
            # pairs); the previous i-tile's o_proj pieces are emitted one at
            # a time between j-chunks so they are available as PE filler
            # during the exp->mask->AV latency chains
            o_pending = []

            def o_proj_piece(sc, ht):
                scs = bass.ts(sc, 128)
                o_ps = acc_ps.tile([128, HT], F32, tag="acc")
                for m in range(qh):
                    nc.tensor.matmul(
                        o_ps, attn_slice(m, scs),
                        wo_sb[:, m, bass.ts(ht, HT)],
                        start=(m == 0), stop=(m == qh - 1),
                    )
                o_sb = outp.tile([128, HT], BF16)
                if (sc + ht) % 2 == 0:
                    nc.scalar.copy(o_sb, o_ps)
                else:
                    nc.vector.tensor_copy(o_sb, o_ps)
                nc.sync.dma_start(
                    out=out[scs, bass.ts(ht, HT)], in_=o_sb
                )

            def emit_o(n):
                for _ in range(n):
                    if o_pending:
                        o_proj_piece(*o_pending.pop(0))

            # attention i-tiles: narrow at the start (shrinks the
            # filler-less warmup) and at the end (shrinks the un-overlapped
            # o_proj tail), wide in the middle; narrower diag tiles also
            # skip more of the causal upper triangle
            ATILES = [(0, 256), (256, 256), (512, 512), (1024, 512),
                      (1536, 512)]

            def attn_tile(hp, i0, width, o_per_jc):
                NC2 = width // 128
                heads = (2 * hp, 2 * hp + 1)
                iss = slice(i0, i0 + width)
                av_ps = [acc_ps.tile([128, width], F32, tag="acc",
                                     name=f"av_ps{i}") for i in range(2)]
                njc = (i0 + width) // 128
                # small warmup tiles: Z via the direct ones-matmul into a
                # [1, w] row (cheap at small njc*w, and the row layout needs
                # no transpose/copies -- a much shorter evac chain).  Large
                # tiles use flipped single-shot matmuls + SBUF accumulation.
                rowz = njc <= 4
                if rowz:
                    zrow = [acc_ps.tile([1, width], F32, tag="acc",
                                        name=f"zrow{i}") for i in range(2)]
                else:
                    scr = acc_ps.tile([128, 320], F32, tag="acc",
                                      name="a_scr")
                    z_acc = statp.tile([128, 8 * NC2], F32, tag="zacc",
                                       name="z_acc")
                    nc.vector.memset(z_acc, 0.0)
                o_carry = 0.0
                nrm = 0      # normal-chunk counter for pair batching
                for jc in range(njc):
                    st_, sp_ = (jc == 0), (jc == njc - 1)
                    rel = jc * 128 - i0
                    diag = rel + 128 > 0
                    # diag chunks only need query columns >= the diagonal;
                    # restrict on wide tiles (bf16 scores run 1 cyc/row at
                    # any width; the f32r AV at 128 wide costs the same 4x)
                    restr = diag and rel > 0
                    if restr:
                        if not rowz and nrm % 2 == 1:
                            # fold the unpaired normal chunk's parity-0
                            # region before restricted chunks reuse it
                            nc.vector.tensor_add(z_acc[:, 0:4 * NC2],
                                                 z_acc[:, 0:4 * NC2],
                                                 scr[:, 0:4 * NC2])
                            nrm += 1
                        nskip = rel // 128
                        for i in range(2):
                            s_ps = acc_ps.tile([128, width], F32, tag="acc",
                                               name=f"s_ps{i}")
                            nc.tensor.matmul(
                                s_ps[:, rel:width],
                                kT_sb[:, bass.ts(jc, 128)],
                                qT_sb[:, heads[i], i0 + rel:i0 + width],
                                start=True, stop=True,
                            )
                            p_sb = probs.tile([128, width], BF16, tag="p",
                                              name=f"p_sb{i}", bufs=10)
                            nc.scalar.activation(
                                p_sb[:, rel:width], s_ps[:, rel:width],
                                mybir.ActivationFunctionType.Exp,
                                scale=scale,
                            )
                            nc.gpsimd.affine_select(
                                out=p_sb[:, rel:rel + 128],
                                in_=p_sb[:, rel:rel + 128],
                                pattern=[[1, 128]],
                                compare_op=mybir.AluOpType.is_ge,
                                fill=0.0, base=0, channel_multiplier=-1,
                            )
                            nc.tensor.matmul(
                                av_ps[i][:, rel:width], vnat_sb[:, jc, :],
                                p_sb[:, rel:width],
                                start=False, stop=sp_,
                                skip_group_check=True,
                            )
                            if rowz:
                                nc.tensor.matmul(
                                    zrow[i][:, rel:width], ones_bf,
                                    p_sb[:, rel:width],
                                    start=False, stop=sp_,
                                    skip_group_check=True,
                                )
                            else:
                                for c in range(nskip, NC2):
                                    zo = 2 * NC2 * i + 2 * c
                                    nc.tensor.matmul(
                                        scr[:, zo:zo + 1],
                                        p_sb[:, bass.ts(c, 128)], ones_bf,
                                        start=True, stop=True,
                                    )
                        if not rowz:
                            # per-chunk fold of the freshly-written slots
                            for i in range(2):
                                zo = 2 * NC2 * i + 2 * nskip
                                hi = 2 * NC2 * (i + 1)
                                nc.vector.tensor_add(
                                    z_acc[:, zo:hi], z_acc[:, zo:hi],
                                    scr[:, zo:hi])
                    else:
                        zoff = 4 * NC2 * (nrm % 2)
                        for i in range(2):
                            s_ps = acc_ps.tile([128, width], F32, tag="acc",
                                               name=f"s_ps{i}")
                            nc.tensor.matmul(
                                s_ps, kT_sb[:, bass.ts(jc, 128)],
                                qT_sb[:, heads[i], iss],
                                start=True, stop=True,
                            )
                            p_sb = probs.tile([128, width], BF16, tag="p",
                                              name=f"p_sb{i}", bufs=10)
                            nc.scalar.activation(
                                p_sb, s_ps,
                                mybir.ActivationFunctionType.Exp,
                                scale=scale,
                            )
                            if diag:
                                nc.gpsimd.affine_select(
                                    out=p_sb, in_=p_sb,
                                    pattern=[[1, width]],
                                    compare_op=mybir.AluOpType.is_ge,
                                    fill=0.0,
                                    base=i0 - jc * 128,
                                    channel_multiplier=-1,
                                )
                            nc.tensor.matmul(av_ps[i], vnat_sb[:, jc, :],
                                             p_sb, start=st_, stop=sp_,
                                             skip_group_check=True)
                            if rowz:
                                nc.tensor.matmul(
                                    zrow[i], ones_bf, p_sb,
                                    start=st_, stop=sp_,
                                    skip_group_check=True,
                                )
                            else:
                                for c in range(NC2):
                                    zo = zoff + 2 * NC2 * i + 2 * c
                                    nc.tensor.matmul(
                                        scr[:, zo:zo + 1],
                                        p_sb[:, bass.ts(c, 128)], ones_bf,
                                        start=True, stop=True,
                                    )
                        if not rowz and nrm % 2 == 1:
                            nc.vector.tensor_add(z_acc, z_acc,
                                                 scr[:, 0:8 * NC2])
                        nrm += 1
                    o_carry += o_per_jc
                    if o_carry >= 1.0:
                        n = int(o_carry)
                        o_carry -= n
                        emit_o(n)
                if not rowz and nrm % 2 == 1:
                    # odd normal count with no restricted chunks after
                    nc.vector.tensor_add(z_acc[:, 0:4 * NC2],
                                         z_acc[:, 0:4 * NC2],
                                         scr[:, 0:4 * NC2])
                if not rowz:
                    z_f = statp.tile([128, 4 * NC2], F32, tag="zacc_f",
                                     name="z_f")
                    nc.vector.tensor_add(z_f, z_acc[:, 0:4 * NC2],
                                         z_acc[:, 4 * NC2:8 * NC2])
                for i, hh in enumerate(heads):
                    if rowz:
                        zf_sb = statp.tile([1, width], F32, tag="statf",
                                           name="zf_sb", bufs=3)
                        nc.vector.reciprocal(zf_sb, zrow[i])
                        ZR_sb = bcastp.tile([128, width], F32, tag="bcast",
                                            name="ZR_sb")
                        nc.gpsimd.partition_broadcast(ZR_sb, zf_sb)
                        nc.vector.tensor_mul(attn_slice(hh, iss), av_ps[i],
                                             ZR_sb)
                        continue
                    zr_sb = statp.tile([128, NC2, 32], F32, tag="stat4b",
                                       name="zr_sb")
                    for c in range(NC2):
                        zo = 2 * NC2 * i + 2 * c
                        nc.vector.reciprocal(
                            zr_sb[:, c, 0:1], z_f[:, zo:zo + 1])
                    zrT_ps = scr[0:32 * NC2, 64 + 128 * i:192 + 128 * i]
                    nc.tensor.transpose(zrT_ps, zr_sb, ident_sb)
                    zf_sb = statp.tile([1, width], F32, tag="statf",
                                       name="zf_sb", bufs=3)
                    for c in range(NC2):
                        nc.vector.tensor_copy(
                            zf_sb[0:1, bass.ts(c, 128)],
                            zrT_ps[32 * c:32 * c + 1, :])
                    ZR_sb = bcastp.tile([128, width], F32, tag="bcast",
                                        name="ZR_sb")
                    nc.gpsimd.partition_broadcast(ZR_sb, zf_sb)
                    nc.vector.tensor_mul(attn_slice(hh, iss), av_ps[i],
                                         ZR_sb)

            if stop_after not in ("p1", "p2"):
                for i0, width in ATILES:
                    # pieces from the previous i-tile, spread across this
                    # tile's 2 * njc j-chunk iterations
                    njc = (i0 + width) // 128
                    o_per_jc = len(o_pending) / (2.0 * njc)
                    for hp in range(qh // 2):
                        attn_tile(hp, i0, width, o_per_jc)
                    if stop_after is None:
                        o_pending.extend(
                            (sc, ht)
                            for sc in range(i0 // 128, (i0 + width) // 128)
                            for ht in range(NHT)
                        )
                emit_o(len(o_pending))

    nc.compile()
    return nc


def make_core_inputs(hidden_states, cos, sin, norm_w, wq, wk, wv, wo,
                     s=S, h=H, qh=QH, n_cores=N_CORES):
    """Host-side sharding + layout preparation. Returns list of in_maps."""
    import ml_dtypes

    bf16 = ml_dtypes.bfloat16
    dq = qh * HD
    dkv = DKV
    x = np.asarray(hidden_states, dtype=np.float32).reshape(s, h)
    nw = np.asarray(norm_w, dtype=np.float32)
    xT = np.ascontiguousarray(x.T.astype(bf16))         # [h, s]
    cosT = np.ascontiguousarray(
        np.asarray(cos, np.float32).reshape(s, HD).T.astype(bf16))
    sinT = np.ascontiguousarray(np.asarray(sin, np.float32).reshape(s, HD).T)
    # swapped/sign-flipped sin table: rows 0:64 = +sin_half, 64:128 = -sin_half
    sin_half = sinT[0:64]
    sinTs = np.ascontiguousarray(
        np.concatenate([sinT[64:128], -sin_half], axis=0).astype(bf16))
    # fold norm_w into the projection weights
    wq_f = np.asarray(wq, np.float32) * nw[:, None]
    wk_f = np.asarray(wk, np.float32) * nw[:, None]
    wv_f = np.asarray(wv, np.float32) * nw[:, None]
    wo_f = np.asarray(wo, np.float32)

    in_maps = []
    for c in range(n_cores):
        in_maps.append({
            "xT": xT,
            "wq": np.ascontiguousarray(wq_f[:, c * dq:(c + 1) * dq].astype(bf16)),
            "wk": np.ascontiguousarray(wk_f[:, c * dkv:(c + 1) * dkv].astype(bf16)),
            "wv": np.ascontiguousarray(wv_f[:, c * dkv:(c + 1) * dkv].astype(bf16)),
            "wo": np.ascontiguousarray(wo_f[c * dq:(c + 1) * dq, :].astype(bf16)),
            "cosT": cosT,
            "sinTs": sinTs,
        })
    return in_maps


_NC_CACHE = {}


def kernel(hidden_states, cos, sin, norm_w, wq, wk, wv, wo):
    from concourse.bass_utils import run_bass_kernel_spmd

    if "nc" not in _NC_CACHE:
        _NC_CACHE["nc"] = build_bass()
    nc = _NC_CACHE["nc"]
    in_maps = make_core_inputs(hidden_states, cos, sin, norm_w, wq, wk, wv, wo)
    res = run_bass_kernel_spmd(nc, in_maps, core_ids=list(range(N_CORES)))
    partials = [m["out"] for m in res.results]
    out = np.asarray(hidden_states, np.float32).reshape(S, H).copy()
    for p in partials:
        out += np.asarray(p, dtype=np.float32)
    return out.reshape(B, S, H)


# revision 71
# speedup vs baseline: 1.4460x; 1.0073x over previous
"""Mixtral attention layer (B=1, S=2048, H=4096, NH=32, NKV=8, HD=128) on 8
Trainium2 NeuronCores, tensor-parallel over heads.

Sharding: core c owns 4 query heads + 1 KV head (column-shard of wq/wk/wv,
row-shard of wo).  Each core computes a full [S, H] partial of the o_proj
output (bf16); the host sums the 8 partials and adds the residual (the
gather of a row-parallel matmul).

Key ideas (PE matmul cost is output-free-size x cycles/row; fp32r at
free>=256 and bf16 run at 1 cycle/row):
  * Partition-dim reductions (RMSNorm sum(x^2), softmax row-sums Z) use
    FLIPPED matmuls: the data tile is the stationary operand and a ones
    vector the moving one, so each costs ~1 output column instead of the
    tile width.  Each flipped matmul is single-shot (start+stop) into a
    per-chunk-pair region of a scratch PSUM bank (concurrently-open
    accumulation groups in one PSUM bank are illegal: a start wipes the
    other open groups); a DVE add folds pairs into an SBUF accumulator.
    The [s-on-partitions] results are rotated back to [1,s] rows with a
    PE transpose (inputs spread to columns 0/32/64/96 so the transposed
    rows land on 32-aligned partition bases, which DVE reads require).
  * x stream and all weights are bf16 (walrus forbids mixing f32r with
    bf16 in one matmul, so pairs are bf16xbf16); the attention
    internals (kT/qT/vnat/probs, all accumulated in fp32 PSUM) stay
    f32r; attnT/wo (o_proj pair) and the output partials are bf16.
  * Phase 1 fuses norm stats into the projection stream (x read once):
    per H-chunk, 6 accumulating projection matmuls + ACT square + 4
    flipped stats matmuls (lagged 4 chunks so tile starts are pure
    projection work).  Weight DMAs are chunked and interleaved with
    tile-0 x chunks so the PE starts ~4us in; wo streams during tiles
    1-3 into its own SBUF slot; v is transposed to natural [j, d]
    layout mid-next-tile.  norm_w folds into the weights on the host,
    the RMSNorm 1/rms into the RoPE cos/sin tables; q^T/k^T go straight
    to persistent SBUF (no DRAM spill).
  * Attention (causal flash-style, per head-pair per 512-wide i-tile):
    scores^T = kT.T @ qT chunkwise, exp on ACT, causal mask via GPSIMD
    affine_select on diagonal blocks, unnormalized AV accumulating in
    PSUM, Z via flipped matmuls; 1/Z applied at AV evacuation.  The
    previous i-tile's o_proj pieces are emitted one at a time between
    j-chunks so the scheduler can fill the exp->mask->AV latency chains
    with o_proj matmuls.
"""

import math

import numpy as np

import concourse.bass as bass
import concourse.tile as tile
from concourse import bacc, mybir
from concourse.masks import make_identity

F32 = mybir.dt.float32
F32R = mybir.dt.float32r
BF16 = mybir.dt.bfloat16

# Full problem dims
B, S, H, NH, NKV, HD = 1, 2048, 4096, 32, 8, 128
EPS = 1e-5
N_CORES = 8
QH = NH // N_CORES          # query heads per core = 4
DQ = QH * HD                # q columns per core = 512
DKV = (NKV // N_CORES) * HD  # kv columns per core = 128


def build_bass(s=S, h=H, qh=QH, stop_after=None):
    """Build the single-core Bass module (same NEFF on all 8 cores)."""
    ST = 512 if s >= 512 else s       # s-tile width (proj + attention i-tiles)
    NST = s // ST                     # number of s-tiles
    HC = h // 128                     # H contraction chunks
    NJ = s // 128                     # j chunks (keys)
    NHT = h // 512 if h >= 512 else 1  # h tiles for o_proj output
    HT = min(512, h)
    WGRP = 4                          # h-chunks per weight DMA group
    dq = qh * HD
    scale = 1.0 / math.sqrt(HD)

    nc = bacc.Bacc(None, target_bir_lowering=False)

    xT = nc.dram_tensor("xT", [h, s], BF16, kind="ExternalInput")
    wq = nc.dram_tensor("wq", [h, dq], BF16, kind="ExternalInput")
    wk = nc.dram_tensor("wk", [h, DKV], BF16, kind="ExternalInput")
    wv = nc.dram_tensor("wv", [h, DKV], BF16, kind="ExternalInput")
    wo = nc.dram_tensor("wo", [dq, h], BF16, kind="ExternalInput")
    cosT = nc.dram_tensor("cosT", [HD, s], BF16, kind="ExternalInput")
    sinTs = nc.dram_tensor("sinTs", [HD, s], BF16, kind="ExternalInput")
    out = nc.dram_tensor("out", [s, h], BF16, kind="ExternalOutput")

    xT_t = xT.rearrange("(ho hi) s -> hi ho s", hi=128)
    wq_t = wq.rearrange("(ho hi) d -> hi ho d", hi=128)
    wk_t = wk.rearrange("(ho hi) d -> hi ho d", hi=128)
    wv_t = wv.rearrange("(ho hi) d -> hi ho d", hi=128)
    wo_t = wo.rearrange("(do di) h -> di do h", di=128)

    with tile.TileContext(nc) as tc:
        with (
            tc.tile_pool(name="persist", bufs=1) as persist,
            tc.tile_pool(name="xin", bufs=4) as xin,
            tc.tile_pool(name="x2b", bufs=5) as x2b,
            tc.tile_pool(name="rope", bufs=3) as ropep,
            tc.tile_pool(name="statp", bufs=6) as statp,
            tc.tile_pool(name="tabp", bufs=3) as tabp,
            tc.tile_pool(name="bcastp", bufs=3) as bcastp,
            tc.tile_pool(name="probs", bufs=6) as probs,
            tc.tile_pool(name="outp", bufs=6) as outp,
            tc.tile_pool(name="acc_ps", bufs=8, space="PSUM") as acc_ps,
        ):
            # ---- persistent SBUF tensors ----
            # Slot reuse chains (same tag, sequential lifetimes):
            #   wq (8MB) -> wo (8MB)         tag "bigw"
            #   wk (2MB) -> attnT heads 0-1  tag "wk"
            #   wv (2MB) -> attnT heads 2-3  tag "wv"
            #   cos (1MB) -> v natural (1MB) tag "cosvnat"
            wq_sb = persist.tile([128, HC, dq], BF16, tag="bigw")
            wk_sb = persist.tile([128, HC, DKV], BF16, tag="wk")
            wv_sb = persist.tile([128, HC, DKV], BF16, tag="wv")
            cos_sb = persist.tile([128, s], BF16, tag="cos")
            sin_sb = persist.tile([128, s], BF16, tag="sin")
            ones_f = persist.tile([128, 1], F32, tag="ones_f")
            ones_sb = persist.tile([128, 2], F32R, tag="ones")
            ones_bf = persist.tile([128, 1], BF16, tag="ones_bf")
            eps_sb = persist.tile([128, 1], F32, tag="eps")
            ident_sb = persist.tile([128, 128], F32, tag="ident")
            kT_sb = persist.tile([128, s], BF16, tag="kT")
            vT_sb = persist.tile([128, s], F32, tag="vT")
            qT_sb = persist.tile([128, qh, s], BF16, tag="qT")
            wo_sb = persist.tile([128, qh, h], BF16, tag="wo")
            vnat_sb = persist.tile([128, NJ, 128], BF16, tag="vnat")

            nc.vector.memset(ones_f, 1.0)
            nc.scalar.copy(ones_sb[:, 0:1], ones_f)
            nc.scalar.copy(ones_sb[:, 1:2], ones_f)
            nc.scalar.copy(ones_bf, ones_f)
            nc.vector.memset(eps_sb, EPS)
            make_identity(nc, ident_sb)

            # ---- phase 1: fused norm stats + q/k/v projections off a single
            # fp32r x^T stream; weight DMAs interleaved with tile-0 x chunks
            def pass_b(st):
                ss = bass.ts(st, ST)
                q_ps = [acc_ps.tile([128, ST], F32, tag="acc", name=f"q_ps{m}")
                        for m in range(qh)]
                k_ps = acc_ps.tile([128, ST], F32, tag="acc", name="k_ps")
                v_ps = acc_ps.tile([128, ST], F32, tag="acc", name="v_ps")
                # sum(x^2) accumulates in SBUF: each chunk's flipped
                # matmuls are single-shot (start+stop) into a transient PSUM
                # tile -- concurrently-open accumulation groups in one PSUM
                # bank are illegal (a start wipes the other open groups)
                # one scratch PSUM bank per s-tile: cols 0:8 hold the
                # per-chunk-pair stats groups, 16:144 the r transpose
                scr = acc_ps.tile([128, 144], F32, tag="acc", name="p1scr")
                sq_acc = statp.tile([128, 8], F32, tag="sqacc",
                                    name="sq_acc")
                nc.vector.memset(sq_acc, 0.0)
                # weight DMA groups for tile 0: small groups first so the
                # PE can start early, then 4-chunk groups
                wgroups = [(0, 1), (1, 1), (2, 2)] + [
                    (g, WGRP) for g in range(4, HC, WGRP)]

                def stats(hc):
                    # flipped stats: sum over h of x^2 lands on the s
                    # partitions; costs ~1 output column per matmul.  Each
                    # matmul is single-shot (concurrently-open accumulation
                    # groups in one PSUM bank are illegal); a chunk pair
                    # lands in scratch cols 0:8, then one DVE add folds it
                    # into the SBUF accumulator.
                    off = 4 * (hc % 2)
                    for c in range(4):
                        nc.tensor.matmul(
                            scr[:, off + c:off + c + 1],
                            x2s[hc][:, bass.ts(c, 128)],
                            ones_bf, start=True, stop=True,
                        )
                    if hc % 2 == 1:
                        nc.vector.tensor_add(sq_acc, sq_acc, scr[:, 0:8])
                    x2s[hc] = None

                x2s = {}
                for hc in range(HC):
                    x_sb = xin.tile([128, ST], BF16)
                    nc.sync.dma_start(out=x_sb, in_=xT_t[:, hc, ss])
                    if st == 0 and wgroups and wgroups[0][0] == hc:
                        g0, gn = wgroups.pop(0)
                        nc.sync.dma_start(out=wq_sb[:, g0:g0 + gn, :],
                                          in_=wq_t[:, g0:g0 + gn, :])
                        nc.sync.dma_start(out=wk_sb[:, g0:g0 + gn, :],
                                          in_=wk_t[:, g0:g0 + gn, :])
                        nc.sync.dma_start(out=wv_sb[:, g0:g0 + gn, :],
                                          in_=wv_t[:, g0:g0 + gn, :])
                    if st > 0 and hc in (6, 14, 22):
                        # o_proj weights trickle in during tiles 1-3
                        ht = 3 * (st - 1) + (hc - 6) // 8
                        if ht < NHT:
                            nc.sync.dma_start(
                                out=wo_sb[:, :, bass.ts(ht, HT)],
                                in_=wo_t[:, :, bass.ts(ht, HT)],
                            )
                    x2_sb = x2b.tile([128, ST], BF16)
                    nc.scalar.square(x2_sb, x_sb)
                    x2s[hc] = x2_sb
                    st_, sp_ = (hc == 0), (hc == HC - 1)
                    for m in range(qh):
                        nc.tensor.matmul(
                            q_ps[m], wq_sb[:, hc, bass.ts(m, 128)], x_sb,
                            start=st_, stop=sp_,
                        )
                    nc.tensor.matmul(k_ps, wk_sb[:, hc, :], x_sb,
                                     start=st_, stop=sp_)
                    nc.tensor.matmul(v_ps, wv_sb[:, hc, :], x_sb,
                                     start=st_, stop=sp_)
                    # stats lag the stream so tile starts are pure
                    # projection work
                    if hc >= 4:
                        stats(hc - 4)
                    # previous s-tile's v -> natural [j, d] transposes,
                    # placed mid-tile where PSUM banks have slack
                    if st > 0 and hc in (16, 18, 20, 22):
                        jc = (st - 1) * (ST // 128) + (hc - 16) // 2
                        vt_ps = acc_ps.tile([128, 128], F32, tag="acc")
                        nc.tensor.transpose(
                            vt_ps, vT_sb[:, bass.ts(jc, 128)], ident_sb)
                        nc.scalar.copy(vnat_sb[:, jc, :], vt_ps)
                if st == 0:
                    # rope tables: after tile-0's weights, before the first
                    # evacuation needs them
                    nc.sync.dma_start(out=cos_sb, in_=cosT[:, :])
                    nc.sync.dma_start(out=sin_sb, in_=sinTs[:, :])
                for hcl in range(HC - 4, HC):
                    stats(hcl)
                # r = 1/sqrt(mean + eps) in [s-part, 4] layout, rotated back
                # to a [1, ST] row for the table broadcast
                sq_f = statp.tile([128, 4], F32, tag="stat4f",
                                  name="sq_f")
                nc.vector.tensor_add(sq_f, sq_acc[:, 0:4], sq_acc[:, 4:8])
                sd_sb = statp.tile([128, 4], F32, tag="stat4",
                                   name="sd_sb")
                nc.scalar.activation(
                    sd_sb, sq_f, mybir.ActivationFunctionType.Sqrt,
                    bias=eps_sb, scale=1.0 / h,
                )
                # reciprocals spread to columns 0/32/64/96 so the
                # transpose lands them on 32-aligned partitions (DVE reads
                # require 32-aligned partition bases)
                rr_sb = statp.tile([128, 4, 32], F32, tag="stat4b",
                                   name="rr_sb")
                for c in range(4):
                    nc.vector.reciprocal(rr_sb[:, c, 0:1], sd_sb[:, c:c + 1])
                rT_ps = scr[:, 16:144]
                nc.tensor.transpose(rT_ps, rr_sb, ident_sb)
                rf_sb = statp.tile([1, ST], BF16, tag="statfb",
                                   name="rf_sb", bufs=3)
                for c in range(4):
                    nc.vector.tensor_copy(
                        rf_sb[0:1, bass.ts(c, 128)],
                        rT_ps[32 * c:32 * c + 1, :])
                R_t = tabp.tile([128, ST], BF16, tag="R", name="R_t")
                nc.gpsimd.partition_broadcast(R_t, rf_sb)
                cp_t = tabp.tile([128, ST], BF16, tag="cp", name="cp_t")
                nc.vector.tensor_mul(cp_t, cos_sb[:, ss], R_t)
                sp_t = tabp.tile([128, ST], BF16, tag="sp", name="sp_t")
                nc.vector.tensor_mul(sp_t, sin_sb[:, ss], R_t)

                # evacuation: fast ACT copy frees the PSUM bank, then
                # norm+RoPE happens SBUF-side on DVE (in place; the u-halves
                # read the raw values before the cos-multiply overwrites)
                # all PSUM->SBUF copies first, alternating ACT/DVE, so
                # the banks free ~2x faster for the next tile; norm+RoPE
                # then happens SBUF-side on DVE (in place; the u-halves
                # read the raw values before the cos-multiply overwrites)
                evacs = [(k_ps, kT_sb[:, ss])]
                evacs += [(q_ps[m], qT_sb[:, m, ss]) for m in range(qh)]
                for idx, (src_ps, dst) in enumerate(evacs):
                    if idx % 2 == 0:
                        nc.scalar.copy(dst, src_ps)
                    else:
                        nc.vector.tensor_copy(dst, src_ps)
                nc.scalar.copy(vT_sb[:, ss], v_ps)

                def rope_rot(dst):
                    u_sb = ropep.tile([128, ST], BF16, tag="u",
                                      name="u_sb", bufs=3)
                    nc.vector.tensor_mul(
                        u_sb[0:64, :], dst[64:128, :], sp_t[64:128, :])
                    nc.vector.tensor_mul(
                        u_sb[64:128, :], dst[0:64, :], sp_t[0:64, :])
                    nc.vector.tensor_mul(dst, dst, cp_t)
                    nc.vector.tensor_add(dst, dst, u_sb)

                for _, dst in evacs:
                    rope_rot(dst)
                nc.vector.tensor_mul(vT_sb[:, ss], vT_sb[:, ss], R_t)

            for st in range(NST):
                pass_b(st)

            # ---- phase 2: last s-tile's v transposes + the last wo
            # chunks (the rest streamed during tiles 1-3) ----
            if stop_after != "p1":
                for ht in range(min(3 * (NST - 1), NHT), NHT):
                    nc.sync.dma_start(
                        out=wo_sb[:, :, bass.ts(ht, HT)],
                        in_=wo_t[:, :, bass.ts(ht, HT)],
                    )
            for jc in range((NST - 1) * (ST // 128),
                            NJ if stop_after != "p1" else 0):
                vt_ps = acc_ps.tile([128, 128], F32, tag="acc")
                nc.tensor.transpose(vt_ps, vT_sb[:, bass.ts(jc, 128)],
                                    ident_sb)
                nc.scalar.copy(vnat_sb[:, jc, :], vt_ps)

            # attn^T reuses the wk/wv slots (heads 0-1 / 2-3)
            attnT_h = [
                persist.tile([128, 2, s], BF16, tag="wk", name="attnT01"),
                persist.tile([128, 2, s], BF16, tag="wv", name="attnT23"),
            ]

            def attn_slice(m, sl):
                return attnT_h[m // 2][:, m % 2, sl]

            # ---- phase 3 + 4 interleaved: attention per i-tile (both head
            # pairs); the previous i-tile's o_proj pieces are emitted one at
            # a time between j-chunks so they are available as PE filler
            # during the exp->mask->AV latency chains
            o_pending = []

            def o_proj_piece(sc, ht):
                scs = bass.ts(sc, 128)
                o_ps = acc_ps.tile([128, HT], F32, tag="acc")
                for m in range(qh):
                    nc.tensor.matmul(
                        o_ps, attn_slice(m, scs),
                        wo_sb[:, m, bass.ts(ht, HT)],
                        start=(m == 0), stop=(m == qh - 1),
                    )
                o_sb = outp.tile([128, HT], BF16)
                if (sc + ht) % 2 == 0:
                    nc.scalar.copy(o_sb, o_ps)
                else:
                    nc.vector.tensor_copy(o_sb, o_ps)
                nc.sync.dma_start(
                    out=out[scs, bass.ts(ht, HT)], in_=o_sb
                )

            def emit_o(n):
                for _ in range(n):
                    if o_pending:
                        o_proj_piece(*o_pending.pop(0))

            # attention i-tiles: narrow at the start (shrinks the
            # filler-less warmup) and at the end (shrinks the un-overlapped
            # o_proj tail), wide in the middle; narrower diag tiles also
            # skip more of the causal upper triangle
            ATILES = [(0, 256), (256, 256), (512, 512), (1024, 512),
                      (1536, 512)]

            def attn_tile(hp, i0, width, o_per_jc):
                NC2 = width // 128
                heads = (2 * hp, 2 * hp + 1)
                iss = slice(i0, i0 + width)
                av_ps = [acc_ps.tile([128, width], F32, tag="acc",
                                     name=f"av_ps{i}") for i in range(2)]
                njc = (i0 + width) // 128
                # small warmup tiles: Z via the direct ones-matmul into a
                # [1, w] row (cheap at small njc*w, and the row layout needs
                # no transpose/copies -- a much shorter evac chain).  Large
                # tiles use flipped single-shot matmuls + SBUF accumulation.
                rowz = njc <= 4
                if rowz:
                    zrow = [acc_ps.tile([1, width], F32, tag="acc",
                                        name=f"zrow{i}") for i in range(2)]
                else:
                    scr = acc_ps.tile([128, 320], F32, tag="acc",
                                      name="a_scr")
                    z_acc = statp.tile([128, 8 * NC2], F32, tag="zacc",
                                       name="z_acc")
                    nc.vector.memset(z_acc, 0.0)
                o_carry = 0.0
                nrm = 0      # normal-chunk counter for pair batching
                for jc in range(njc):
                    st_, sp_ = (jc == 0), (jc == njc - 1)
                    rel = jc * 128 - i0
                    diag = rel + 128 > 0
                    # diag chunks only need query columns >= the diagonal;
                    # restrict on wide tiles (bf16 scores run 1 cyc/row at
                    # any width; the f32r AV at 128 wide costs the same 4x)
                    restr = diag and rel > 0
                    if restr:
                        if not rowz and nrm % 2 == 1:
                            # fold the unpaired normal chunk's parity-0
                            # region before restricted chunks reuse it
                            nc.vector.tensor_add(z_acc[:, 0:4 * NC2],
                                                 z_acc[:, 0:4 * NC2],
                                                 scr[:, 0:4 * NC2])
                            nrm += 1
                        nskip = rel // 128
                        for i in range(2):
                            s_ps = acc_ps.tile([128, width], F32, tag="acc",
                                               name=f"s_ps{i}")
                            nc.tensor.matmul(
                                s_ps[:, rel:width],
                                kT_sb[:, bass.ts(jc, 128)],
                                qT_sb[:, heads[i], i0 + rel:i0 + width],
                                start=True, stop=True,
                            )
                            p_sb = probs.tile([128, width], BF16, tag="p",
                                              name=f"p_sb{i}", bufs=10)
                            nc.scalar.activation(
                                p_sb[:, rel:width], s_ps[:, rel:width],
                                mybir.ActivationFunctionType.Exp,
                                scale=scale,
                            )
                            nc.gpsimd.affine_select(
                                out=p_sb[:, rel:rel + 128],
                                in_=p_sb[:, rel:rel + 128],
                                pattern=[[1, 128]],
                                compare_op=mybir.AluOpType.is_ge,
                                fill=0.0, base=0, channel_multiplier=-1,
                            )
                            nc.tensor.matmul(
                                av_ps[i][:, rel:width], vnat_sb[:, jc, :],
                                p_sb[:, rel:width],
                                start=False, stop=sp_,
                                skip_group_check=True,
                            )
                            if rowz:
                                nc.tensor.matmul(
                                    zrow[i][:, rel:width], ones_bf,
                                    p_sb[:, rel:width],
                                    start=False, stop=sp_,
                                    skip_group_check=True,
                                )
                            else:
                                for c in range(nskip, NC2):
                                    zo = 2 * NC2 * i + 2 * c
                                    nc.tensor.matmul(
                                        scr[:, zo:zo + 1],
                                        p_sb[:, bass.ts(c, 128)], ones_bf,
                                        start=True, stop=True,
                                    )
                        if not rowz:
                            # per-chunk fold of the freshly-written slots
                            for i in range(2):
                                zo = 2 * NC2 * i + 2 * nskip
                                hi = 2 * NC2 * (i + 1)
                                nc.vector.tensor_add(
                                    z_acc[:, zo:hi], z_acc[:, zo:hi],
                                    scr[:, zo:hi])
                    else:
                        zoff = 4 * NC2 * (nrm % 2)
                        for i in range(2):
                            s_ps = acc_ps.tile([128, width], F32, tag="acc",
                                               name=f"s_ps{i}")
                            nc.tensor.matmul(
                                s_ps, kT_sb[:, bass.ts(jc, 128)],
                                qT_sb[:, heads[i], iss],
                                start=True, stop=True,
                            )
                            p_sb = probs.tile([128, width], BF16, tag="p",
                                              name=f"p_sb{i}", bufs=10)
                            nc.scalar.activation(
                                p_sb, s_ps,
                                mybir.ActivationFunctionType.Exp,
                                scale=scale,
                            )
                            if diag:
                                # rel == 0 here: only the first 128-col
                                # block is triangular; later columns are
                                # always kept, so mask just that block
                                nc.gpsimd.affine_select(
                                    out=p_sb[:, 0:128], in_=p_sb[:, 0:128],
                                    pattern=[[1, 128]],
                                    compare_op=mybir.AluOpType.is_ge,
                                    fill=0.0,
                                    base=0,
                                    channel_multiplier=-1,
                                )
                            nc.tensor.matmul(av_ps[i], vnat_sb[:, jc, :],
                                             p_sb, start=st_, stop=sp_,
                                             skip_group_check=True)
                            if rowz:
                                nc.tensor.matmul(
                                    zrow[i], ones_bf, p_sb,
                                    start=st_, stop=sp_,
                                    skip_group_check=True,
                                )
                            else:
                                for c in range(NC2):
                                    zo = zoff + 2 * NC2 * i + 2 * c
                                    nc.tensor.matmul(
                                        scr[:, zo:zo + 1],
                                        p_sb[:, bass.ts(c, 128)], ones_bf,
                                        start=True, stop=True,
                                    )
                        if not rowz and nrm % 2 == 1:
                            nc.vector.tensor_add(z_acc, z_acc,
                                                 scr[:, 0:8 * NC2])
                        nrm += 1
                    o_carry += o_per_jc
                    if o_carry >= 1.0:
                        n = int(o_carry)
                        o_carry -= n
                        emit_o(n)
                if not rowz and nrm % 2 == 1:
                    # odd normal count with no restricted chunks after
                    nc.vector.tensor_add(z_acc[:, 0:4 * NC2],
                                         z_acc[:, 0:4 * NC2],
                                         scr[:, 0:4 * NC2])
                if not rowz:
                    z_f = statp.tile([128, 4 * NC2], F32, tag="zacc_f",
                                     name="z_f")
                    nc.vector.tensor_add(z_f, z_acc[:, 0:4 * NC2],
                                         z_acc[:, 4 * NC2:8 * NC2])
                for i, hh in enumerate(heads):
                    if rowz:
                        zf_sb = statp.tile([1, width], F32, tag="statf",
                                           name="zf_sb", bufs=3)
                        nc.vector.reciprocal(zf_sb, zrow[i])
                        ZR_sb = bcastp.tile([128, width], F32, tag="bcast",
                                            name="ZR_sb")
                        nc.gpsimd.partition_broadcast(ZR_sb, zf_sb)
                        nc.vector.tensor_mul(attn_slice(hh, iss), av_ps[i],
                                             ZR_sb)
                        continue
                    zr_sb = statp.tile([128, NC2, 32], F32, tag="stat4b",
                                       name="zr_sb")
                    for c in range(NC2):
                        zo = 2 * NC2 * i + 2 * c
                        nc.vector.reciprocal(
                            zr_sb[:, c, 0:1], z_f[:, zo:zo + 1])
                    zrT_ps = scr[0:32 * NC2, 64 + 128 * i:192 + 128 * i]
                    nc.tensor.transpose(zrT_ps, zr_sb, ident_sb)
                    zf_sb = statp.tile([1, width], F32, tag="statf",
                                       name="zf_sb", bufs=3)
                    for c in range(NC2):
                        nc.vector.tensor_copy(
                            zf_sb[0:1, bass.ts(c, 128)],
                            zrT_ps[32 * c:32 * c + 1, :])
                    ZR_sb = bcastp.tile([128, width], F32, tag="bcast",
                                        name="ZR_sb")
                    nc.gpsimd.partition_broadcast(ZR_sb, zf_sb)
                    nc.vector.tensor_mul(attn_slice(hh, iss), av_ps[i],
                                         ZR_sb)

            if stop_after not in ("p1", "p2"):
                for i0, width in ATILES:
                    # pieces from the previous i-tile, spread across this
                    # tile's 2 * njc j-chunk iterations
                    njc = (i0 + width) // 128
                    o_per_jc = len(o_pending) / (2.0 * njc)
                    for hp in range(qh // 2):
                        attn_tile(hp, i0, width, o_per_jc)
                    if stop_after is None:
                        o_pending.extend(
                            (sc, ht)
                            for sc in range(i0 // 128, (i0 + width) // 128)
                            for ht in range(NHT)
                        )
                emit_o(len(o_pending))

    nc.compile()
    return nc


def make_core_inputs(hidden_states, cos, sin, norm_w, wq, wk, wv, wo,
                     s=S, h=H, qh=QH, n_cores=N_CORES):
    """Host-side sharding + layout preparation. Returns list of in_maps."""
    import ml_dtypes

    bf16 = ml_dtypes.bfloat16
    dq = qh * HD
    dkv = DKV
    x = np.asarray(hidden_states, dtype=np.float32).reshape(s, h)
    nw = np.asarray(norm_w, dtype=np.float32)
    xT = np.ascontiguousarray(x.T.astype(bf16))         # [h, s]
    cosT = np.ascontiguousarray(
        np.asarray(cos, np.float32).reshape(s, HD).T.astype(bf16))
    sinT = np.ascontiguousarray(np.asarray(sin, np.float32).reshape(s, HD).T)
    # swapped/sign-flipped sin table: rows 0:64 = +sin_half, 64:128 = -sin_half
    sin_half = sinT[0:64]
    sinTs = np.ascontiguousarray(
        np.concatenate([sinT[64:128], -sin_half], axis=0).astype(bf16))
    # fold norm_w into the projection weights
    wq_f = np.asarray(wq, np.float32) * nw[:, None]
    wk_f = np.asarray(wk, np.float32) * nw[:, None]
    wv_f = np.asarray(wv, np.float32) * nw[:, None]
    wo_f = np.asarray(wo, np.float32)

    in_maps = []
    for c in range(n_cores):
        in_maps.append({
            "xT": xT,
            "wq": np.ascontiguousarray(wq_f[:, c * dq:(c + 1) * dq].astype(bf16)),
            "wk": np.ascontiguousarray(wk_f[:, c * dkv:(c + 1) * dkv].astype(bf16)),
            "wv": np.ascontiguousarray(wv_f[:, c * dkv:(c + 1) * dkv].astype(bf16)),
            "wo": np.ascontiguousarray(wo_f[c * dq:(c + 1) * dq, :].astype(bf16)),
            "cosT": cosT,
            "sinTs": sinTs,
        })
    return in_maps


_NC_CACHE = {}


def kernel(hidden_states, cos, sin, norm_w, wq, wk, wv, wo):
    from concourse.bass_utils import run_bass_kernel_spmd

    if "nc" not in _NC_CACHE:
        _NC_CACHE["nc"] = build_bass()
    nc = _NC_CACHE["nc"]
    in_maps = make_core_inputs(hidden_states, cos, sin, norm_w, wq, wk, wv, wo)
    res = run_bass_kernel_spmd(nc, in_maps, core_ids=list(range(N_CORES)))
    partials = [m["out"] for m in res.results]
    out = np.asarray(hidden_states, np.float32).reshape(S, H).copy()
    for p in partials:
        out += np.asarray(p, dtype=np.float32)
    return out.reshape(B, S, H)


# revision 73
# speedup vs baseline: 1.4557x; 1.0067x over previous
"""Mixtral attention layer (B=1, S=2048, H=4096, NH=32, NKV=8, HD=128) on 8
Trainium2 NeuronCores, tensor-parallel over heads.

Sharding: core c owns 4 query heads + 1 KV head (column-shard of wq/wk/wv,
row-shard of wo).  Each core computes a full [S, H] partial of the o_proj
output (bf16); the host sums the 8 partials and adds the residual (the
gather of a row-parallel matmul).

Key ideas (PE matmul cost is output-free-size x cycles/row; fp32r at
free>=256 and bf16 run at 1 cycle/row):
  * Partition-dim reductions (RMSNorm sum(x^2), softmax row-sums Z) use
    FLIPPED matmuls: the data tile is the stationary operand and a ones
    vector the moving one, so each costs ~1 output column instead of the
    tile width.  Each flipped matmul is single-shot (start+stop) into a
    per-chunk-pair region of a scratch PSUM bank (concurrently-open
    accumulation groups in one PSUM bank are illegal: a start wipes the
    other open groups); a DVE add folds pairs into an SBUF accumulator.
    The [s-on-partitions] results are rotated back to [1,s] rows with a
    PE transpose (inputs spread to columns 0/32/64/96 so the transposed
    rows land on 32-aligned partition bases, which DVE reads require).
  * x stream and all weights are bf16 (walrus forbids mixing f32r with
    bf16 in one matmul, so pairs are bf16xbf16); the attention
    internals (kT/qT/vnat/probs, all accumulated in fp32 PSUM) stay
    f32r; attnT/wo (o_proj pair) and the output partials are bf16.
  * Phase 1 fuses norm stats into the projection stream (x read once):
    per H-chunk, 6 accumulating projection matmuls + ACT square + 4
    flipped stats matmuls (lagged 4 chunks so tile starts are pure
    projection work).  Weight DMAs are chunked and interleaved with
    tile-0 x chunks so the PE starts ~4us in; wo streams during tiles
    1-3 into its own SBUF slot; v is transposed to natural [j, d]
    layout mid-next-tile.  norm_w folds into the weights on the host,
    the RMSNorm 1/rms into the RoPE cos/sin tables; q^T/k^T go straight
    to persistent SBUF (no DRAM spill).
  * Attention (causal flash-style, per head-pair per 512-wide i-tile):
    scores^T = kT.T @ qT chunkwise, exp on ACT, causal mask via GPSIMD
    affine_select on diagonal blocks, unnormalized AV accumulating in
    PSUM, Z via flipped matmuls; 1/Z applied at AV evacuation.  The
    previous i-tile's o_proj pieces are emitted one at a time between
    j-chunks so the scheduler can fill the exp->mask->AV latency chains
    with o_proj matmuls.
"""

import math

import numpy as np

import concourse.bass as bass
import concourse.tile as tile
from concourse import bacc, mybir
from concourse.masks import make_identity

F32 = mybir.dt.float32
F32R = mybir.dt.float32r
BF16 = mybir.dt.bfloat16

# Full problem dims
B, S, H, NH, NKV, HD = 1, 2048, 4096, 32, 8, 128
EPS = 1e-5
N_CORES = 8
QH = NH // N_CORES          # query heads per core = 4
DQ = QH * HD                # q columns per core = 512
DKV = (NKV // N_CORES) * HD  # kv columns per core = 128


def build_bass(s=S, h=H, qh=QH, stop_after=None):
    """Build the single-core Bass module (same NEFF on all 8 cores)."""
    ST = 512 if s >= 512 else s       # s-tile width (proj + attention i-tiles)
    NST = s // ST                     # number of s-tiles
    HC = h // 128                     # H contraction chunks
    NJ = s // 128                     # j chunks (keys)
    NHT = h // 512 if h >= 512 else 1  # h tiles for o_proj output
    HT = min(512, h)
    WGRP = 4                          # h-chunks per weight DMA group
    dq = qh * HD
    scale = 1.0 / math.sqrt(HD)

    nc = bacc.Bacc(None, target_bir_lowering=False)

    xT = nc.dram_tensor("xT", [h, s], BF16, kind="ExternalInput")
    wq = nc.dram_tensor("wq", [h, dq], BF16, kind="ExternalInput")
    wk = nc.dram_tensor("wk", [h, DKV], BF16, kind="ExternalInput")
    wv = nc.dram_tensor("wv", [h, DKV], BF16, kind="ExternalInput")
    wo = nc.dram_tensor("wo", [dq, h], BF16, kind="ExternalInput")
    cosT = nc.dram_tensor("cosT", [HD, s], BF16, kind="ExternalInput")
    sinTs = nc.dram_tensor("sinTs", [HD, s], BF16, kind="ExternalInput")
    out = nc.dram_tensor("out", [s, h], BF16, kind="ExternalOutput")

    xT_t = xT.rearrange("(ho hi) s -> hi ho s", hi=128)
    wq_t = wq.rearrange("(ho hi) d -> hi ho d", hi=128)
    wk_t = wk.rearrange("(ho hi) d -> hi ho d", hi=128)
    wv_t = wv.rearrange("(ho hi) d -> hi ho d", hi=128)
    wo_t = wo.rearrange("(do di) h -> di do h", di=128)

    with tile.TileContext(nc) as tc:
        with (
            tc.tile_pool(name="persist", bufs=1) as persist,
            tc.tile_pool(name="xin", bufs=4) as xin,
            tc.tile_pool(name="x2b", bufs=7) as x2b,
            tc.tile_pool(name="rope", bufs=3) as ropep,
            tc.tile_pool(name="statp", bufs=6) as statp,
            tc.tile_pool(name="tabp", bufs=3) as tabp,
            tc.tile_pool(name="bcastp", bufs=3) as bcastp,
            tc.tile_pool(name="probs", bufs=6) as probs,
            tc.tile_pool(name="outp", bufs=6) as outp,
            tc.tile_pool(name="acc_ps", bufs=8, space="PSUM") as acc_ps,
        ):
            # ---- persistent SBUF tensors ----
            # Slot reuse chains (same tag, sequential lifetimes):
            #   wq (8MB) -> wo (8MB)         tag "bigw"
            #   wk (2MB) -> attnT heads 0-1  tag "wk"
            #   wv (2MB) -> attnT heads 2-3  tag "wv"
            #   cos (1MB) -> v natural (1MB) tag "cosvnat"
            wq_sb = persist.tile([128, HC, dq], BF16, tag="bigw")
            wk_sb = persist.tile([128, HC, DKV], BF16, tag="wk")
            wv_sb = persist.tile([128, HC, DKV], BF16, tag="wv")
            cos_sb = persist.tile([128, s], BF16, tag="cos")
            sin_sb = persist.tile([128, s], BF16, tag="sin")
            ones_f = persist.tile([128, 1], F32, tag="ones_f")
            ones_sb = persist.tile([128, 2], F32R, tag="ones")
            ones_bf = persist.tile([128, 1], BF16, tag="ones_bf")
            eps_sb = persist.tile([128, 1], F32, tag="eps")
            ident_sb = persist.tile([128, 128], F32, tag="ident")
            kT_sb = persist.tile([128, s], BF16, tag="kT")
            vT_sb = persist.tile([128, s], F32, tag="vT")
            qT_sb = persist.tile([128, qh, s], BF16, tag="qT")
            wo_sb = persist.tile([128, qh, h], BF16, tag="wo")
            vnat_sb = persist.tile([128, NJ, 128], BF16, tag="vnat")

            nc.vector.memset(ones_f, 1.0)
            nc.scalar.copy(ones_sb[:, 0:1], ones_f)
            nc.scalar.copy(ones_sb[:, 1:2], ones_f)
            nc.scalar.copy(ones_bf, ones_f)
            nc.vector.memset(eps_sb, EPS)
            make_identity(nc, ident_sb)

            # ---- phase 1: fused norm stats + q/k/v projections off a single
            # fp32r x^T stream; weight DMAs interleaved with tile-0 x chunks
            def pass_b(st):
                ss = bass.ts(st, ST)
                q_ps = [acc_ps.tile([128, ST], F32, tag="acc", name=f"q_ps{m}")
                        for m in range(qh)]
                k_ps = acc_ps.tile([128, ST], F32, tag="acc", name="k_ps")
                v_ps = acc_ps.tile([128, ST], F32, tag="acc", name="v_ps")
                # sum(x^2) accumulates in SBUF: each chunk's flipped
                # matmuls are single-shot (start+stop) into a transient PSUM
                # tile -- concurrently-open accumulation groups in one PSUM
                # bank are illegal (a start wipes the other open groups)
                # one scratch PSUM bank per s-tile: cols 0:8 hold the
                # per-chunk-pair stats groups, 16:144 the r transpose
                scr = acc_ps.tile([128, 144], F32, tag="acc", name="p1scr")
                sq_acc = statp.tile([128, 8], F32, tag="sqacc",
                                    name="sq_acc")
                nc.vector.memset(sq_acc, 0.0)
                # weight DMA groups for tile 0: small groups first so the
                # PE can start early, then 4-chunk groups
                wgroups = [(0, 1), (1, 1), (2, 2)] + [
                    (g, WGRP) for g in range(4, HC, WGRP)]

                def stats(hc):
                    # flipped stats: sum over h of x^2 lands on the s
                    # partitions; costs ~1 output column per matmul.  Each
                    # matmul is single-shot (concurrently-open accumulation
                    # groups in one PSUM bank are illegal); a chunk pair
                    # lands in scratch cols 0:8, then one DVE add folds it
                    # into the SBUF accumulator.
                    off = 4 * (hc % 2)
                    for c in range(4):
                        nc.tensor.matmul(
                            scr[:, off + c:off + c + 1],
                            x2s[hc][:, bass.ts(c, 128)],
                            ones_bf, start=True, stop=True,
                        )
                    if hc % 2 == 1:
                        nc.vector.tensor_add(sq_acc, sq_acc, scr[:, 0:8])
                    x2s[hc] = None

                x2s = {}
                for hc in range(HC):
                    x_sb = xin.tile([128, ST], BF16)
                    nc.sync.dma_start(out=x_sb, in_=xT_t[:, hc, ss])
                    if st == 0 and wgroups and wgroups[0][0] == hc:
                        g0, gn = wgroups.pop(0)
                        nc.sync.dma_start(out=wq_sb[:, g0:g0 + gn, :],
                                          in_=wq_t[:, g0:g0 + gn, :])
                        nc.sync.dma_start(out=wk_sb[:, g0:g0 + gn, :],
                                          in_=wk_t[:, g0:g0 + gn, :])
                        nc.sync.dma_start(out=wv_sb[:, g0:g0 + gn, :],
                                          in_=wv_t[:, g0:g0 + gn, :])
                    if st > 0 and hc in (6, 14, 22):
                        # o_proj weights trickle in during tiles 1-3
                        ht = 3 * (st - 1) + (hc - 6) // 8
                        if ht < NHT:
                            nc.sync.dma_start(
                                out=wo_sb[:, :, bass.ts(ht, HT)],
                                in_=wo_t[:, :, bass.ts(ht, HT)],
                            )
                    x2_sb = x2b.tile([128, ST], BF16)
                    nc.scalar.square(x2_sb, x_sb)
                    x2s[hc] = x2_sb
                    st_, sp_ = (hc == 0), (hc == HC - 1)
                    for m in range(qh):
                        nc.tensor.matmul(
                            q_ps[m], wq_sb[:, hc, bass.ts(m, 128)], x_sb,
                            start=st_, stop=sp_,
                        )
                    nc.tensor.matmul(k_ps, wk_sb[:, hc, :], x_sb,
                                     start=st_, stop=sp_)
                    nc.tensor.matmul(v_ps, wv_sb[:, hc, :], x_sb,
                                     start=st_, stop=sp_)
                    # stats lag the stream so tile starts are pure
                    # projection work
                    if hc >= 6:
                        stats(hc - 6)
                    # previous s-tile's v -> natural [j, d] transposes,
                    # placed mid-tile where PSUM banks have slack
                    if st > 0 and hc in (16, 18, 20, 22):
                        jc = (st - 1) * (ST // 128) + (hc - 16) // 2
                        vt_ps = acc_ps.tile([128, 128], F32, tag="acc")
                        nc.tensor.transpose(
                            vt_ps, vT_sb[:, bass.ts(jc, 128)], ident_sb)
                        nc.scalar.copy(vnat_sb[:, jc, :], vt_ps)
                if st == 0:
                    # rope tables: after tile-0's weights, before the first
                    # evacuation needs them
                    nc.sync.dma_start(out=cos_sb, in_=cosT[:, :])
                    nc.sync.dma_start(out=sin_sb, in_=sinTs[:, :])
                for hcl in range(HC - 6, HC):
                    stats(hcl)
                # r = 1/sqrt(mean + eps) in [s-part, 4] layout, rotated back
                # to a [1, ST] row for the table broadcast
                sq_f = statp.tile([128, 4], F32, tag="stat4f",
                                  name="sq_f")
                nc.vector.tensor_add(sq_f, sq_acc[:, 0:4], sq_acc[:, 4:8])
                sd_sb = statp.tile([128, 4], F32, tag="stat4",
                                   name="sd_sb")
                nc.scalar.activation(
                    sd_sb, sq_f, mybir.ActivationFunctionType.Sqrt,
                    bias=eps_sb, scale=1.0 / h,
                )
                # reciprocals spread to columns 0/32/64/96 so the
                # transpose lands them on 32-aligned partitions (DVE reads
                # require 32-aligned partition bases)
                rr_sb = statp.tile([128, 4, 32], F32, tag="stat4b",
                                   name="rr_sb")
                for c in range(4):
                    nc.vector.reciprocal(rr_sb[:, c, 0:1], sd_sb[:, c:c + 1])
                rT_ps = scr[:, 16:144]
                nc.tensor.transpose(rT_ps, rr_sb, ident_sb)
                rf_sb = statp.tile([1, ST], BF16, tag="statfb",
                                   name="rf_sb", bufs=3)
                for c in range(4):
                    nc.vector.tensor_copy(
                        rf_sb[0:1, bass.ts(c, 128)],
                        rT_ps[32 * c:32 * c + 1, :])
                R_t = tabp.tile([128, ST], BF16, tag="R", name="R_t")
                nc.gpsimd.partition_broadcast(R_t, rf_sb)
                cp_t = tabp.tile([128, ST], BF16, tag="cp", name="cp_t")
                nc.vector.tensor_mul(cp_t, cos_sb[:, ss], R_t)
                sp_t = tabp.tile([128, ST], BF16, tag="sp", name="sp_t")
                nc.vector.tensor_mul(sp_t, sin_sb[:, ss], R_t)

                # evacuation: fast ACT copy frees the PSUM bank, then
                # norm+RoPE happens SBUF-side on DVE (in place; the u-halves
                # read the raw values before the cos-multiply overwrites)
                # all PSUM->SBUF copies first, alternating ACT/DVE, so
                # the banks free ~2x faster for the next tile; norm+RoPE
                # then happens SBUF-side on DVE (in place; the u-halves
                # read the raw values before the cos-multiply overwrites)
                evacs = [(k_ps, kT_sb[:, ss])]
                evacs += [(q_ps[m], qT_sb[:, m, ss]) for m in range(qh)]
                for idx, (src_ps, dst) in enumerate(evacs):
                    if idx % 2 == 0:
                        nc.scalar.copy(dst, src_ps)
                    else:
                        nc.vector.tensor_copy(dst, src_ps)
                nc.scalar.copy(vT_sb[:, ss], v_ps)

                def rope_rot(dst):
                    u_sb = ropep.tile([128, ST], BF16, tag="u",
                                      name="u_sb", bufs=3)
                    nc.vector.tensor_mul(
                        u_sb[0:64, :], dst[64:128, :], sp_t[64:128, :])
                    nc.vector.tensor_mul(
                        u_sb[64:128, :], dst[0:64, :], sp_t[0:64, :])
                    nc.vector.tensor_mul(dst, dst, cp_t)
                    nc.vector.tensor_add(dst, dst, u_sb)

                for _, dst in evacs:
                    rope_rot(dst)
                nc.vector.tensor_mul(vT_sb[:, ss], vT_sb[:, ss], R_t)

            for st in range(NST):
                pass_b(st)

            # ---- phase 2: last s-tile's v transposes + the last wo
            # chunks (the rest streamed during tiles 1-3) ----
            if stop_after != "p1":
                for ht in range(min(3 * (NST - 1), NHT), NHT):
                    nc.sync.dma_start(
                        out=wo_sb[:, :, bass.ts(ht, HT)],
                        in_=wo_t[:, :, bass.ts(ht, HT)],
                    )
            for jc in range((NST - 1) * (ST // 128),
                            NJ if stop_after != "p1" else 0):
                vt_ps = acc_ps.tile([128, 128], F32, tag="acc")
                nc.tensor.transpose(vt_ps, vT_sb[:, bass.ts(jc, 128)],
                                    ident_sb)
                nc.scalar.copy(vnat_sb[:, jc, :], vt_ps)

            # attn^T reuses the wk/wv slots (heads 0-1 / 2-3)
            attnT_h = [
                persist.tile([128, 2, s], BF16, tag="wk", name="attnT01"),
                persist.tile([128, 2, s], BF16, tag="wv", name="attnT23"),
            ]

            def attn_slice(m, sl):
                return attnT_h[m // 2][:, m % 2, sl]

            # ---- phase 3 + 4 interleaved: attention per i-tile (both head
            # pairs); the previous i-tile's o_proj pieces are emitted one at
            # a time between j-chunks so they are available as PE filler
            # during the exp->mask->AV latency chains
            o_pending = []

            def o_proj_piece(sc, ht):
                scs = bass.ts(sc, 128)
                o_ps = acc_ps.tile([128, HT], F32, tag="acc")
                for m in range(qh):
                    nc.tensor.matmul(
                        o_ps, attn_slice(m, scs),
                        wo_sb[:, m, bass.ts(ht, HT)],
                        start=(m == 0), stop=(m == qh - 1),
                    )
                o_sb = outp.tile([128, HT], BF16)
                if (sc + ht) % 2 == 0:
                    nc.scalar.copy(o_sb, o_ps)
                else:
                    nc.vector.tensor_copy(o_sb, o_ps)
                nc.sync.dma_start(
                    out=out[scs, bass.ts(ht, HT)], in_=o_sb
                )

            def emit_o(n):
                for _ in range(n):
                    if o_pending:
                        o_proj_piece(*o_pending.pop(0))

            # attention i-tiles: narrow at the start (shrinks the
            # filler-less warmup) and at the end (shrinks the un-overlapped
            # o_proj tail), wide in the middle; narrower diag tiles also
            # skip more of the causal upper triangle
            ATILES = [(0, 256), (256, 256), (512, 512), (1024, 512),
                      (1536, 512)]

            def attn_tile(hp, i0, width, o_per_jc):
                NC2 = width // 128
                heads = (2 * hp, 2 * hp + 1)
                iss = slice(i0, i0 + width)
                av_ps = [acc_ps.tile([128, width], F32, tag="acc",
                                     name=f"av_ps{i}") for i in range(2)]
                njc = (i0 + width) // 128
                # small warmup tiles: Z via the direct ones-matmul into a
                # [1, w] row (cheap at small njc*w, and the row layout needs
                # no transpose/copies -- a much shorter evac chain).  Large
                # tiles use flipped single-shot matmuls + SBUF accumulation.
                rowz = njc <= 4
                if rowz:
                    zrow = [acc_ps.tile([1, width], F32, tag="acc",
                                        name=f"zrow{i}") for i in range(2)]
                else:
                    scr = acc_ps.tile([128, 320], F32, tag="acc",
                                      name="a_scr")
                    z_acc = statp.tile([128, 8 * NC2], F32, tag="zacc",
                                       name="z_acc")
                    nc.vector.memset(z_acc, 0.0)
                o_carry = 0.0
                nrm = 0      # normal-chunk counter for pair batching
                for jc in range(njc):
                    st_, sp_ = (jc == 0), (jc == njc - 1)
                    rel = jc * 128 - i0
                    diag = rel + 128 > 0
                    # diag chunks only need query columns >= the diagonal;
                    # restrict on wide tiles (bf16 scores run 1 cyc/row at
                    # any width; the f32r AV at 128 wide costs the same 4x)
                    restr = diag and rel > 0
                    if restr:
                        if not rowz and nrm % 2 == 1:
                            # fold the unpaired normal chunk's parity-0
                            # region before restricted chunks reuse it
                            nc.vector.tensor_add(z_acc[:, 0:4 * NC2],
                                                 z_acc[:, 0:4 * NC2],
                                                 scr[:, 0:4 * NC2])
                            nrm += 1
                        nskip = rel // 128
                        for i in range(2):
                            s_ps = acc_ps.tile([128, width], F32, tag="acc",
                                               name=f"s_ps{i}")
                            nc.tensor.matmul(
                                s_ps[:, rel:width],
                                kT_sb[:, bass.ts(jc, 128)],
                                qT_sb[:, heads[i], i0 + rel:i0 + width],
                                start=True, stop=True,
                            )
                            p_sb = probs.tile([128, width], BF16, tag="p",
                                              name=f"p_sb{i}", bufs=10)
                            nc.scalar.activation(
                                p_sb[:, rel:width], s_ps[:, rel:width],
                                mybir.ActivationFunctionType.Exp,
                                scale=scale,
                            )
                            nc.gpsimd.affine_select(
                                out=p_sb[:, rel:rel + 128],
                                in_=p_sb[:, rel:rel + 128],
                                pattern=[[1, 128]],
                                compare_op=mybir.AluOpType.is_ge,
                                fill=0.0, base=0, channel_multiplier=-1,
                            )
                            nc.tensor.matmul(
                                av_ps[i][:, rel:width], vnat_sb[:, jc, :],
                                p_sb[:, rel:width],
                                start=False, stop=sp_,
                                skip_group_check=True,
                            )
                            if rowz:
                                nc.tensor.matmul(
                                    zrow[i][:, rel:width], ones_bf,
                                    p_sb[:, rel:width],
                                    start=False, stop=sp_,
                                    skip_group_check=True,
                                )
                            else:
                                for c in range(nskip, NC2):
                                    zo = 2 * NC2 * i + 2 * c
                                    nc.tensor.matmul(
                                        scr[:, zo:zo + 1],
                                        p_sb[:, bass.ts(c, 128)], ones_bf,
                                        start=True, stop=True,
                                    )
                        if not rowz:
                            # per-chunk fold of the freshly-written slots
                            for i in range(2):
                                zo = 2 * NC2 * i + 2 * nskip
                                hi = 2 * NC2 * (i + 1)
                                nc.vector.tensor_add(
                                    z_acc[:, zo:hi], z_acc[:, zo:hi],
                                    scr[:, zo:hi])
                    else:
                        zoff = 4 * NC2 * (nrm % 2)
                        for i in range(2):
                            s_ps = acc_ps.tile([128, width], F32, tag="acc",
                                               name=f"s_ps{i}")
                            nc.tensor.matmul(
                                s_ps, kT_sb[:, bass.ts(jc, 128)],
                                qT_sb[:, heads[i], iss],
                                start=True, stop=True,
                            )
                            p_sb = probs.tile([128, width], BF16, tag="p",
                                              name=f"p_sb{i}", bufs=10)
                            nc.scalar.activation(
                                p_sb, s_ps,
                                mybir.ActivationFunctionType.Exp,
                                scale=scale,
                            )
                            if diag:
                                # rel == 0 here: only the first 128-col
                                # block is triangular; later columns are
                                # always kept, so mask just that block
                                nc.gpsimd.affine_select(
                                    out=p_sb[:, 0:128], in_=p_sb[:, 0:128],
                                    pattern=[[1, 128]],
                                    compare_op=mybir.AluOpType.is_ge,
                                    fill=0.0,
                                    base=0,
                                    channel_multiplier=-1,
                                )
                            nc.tensor.matmul(av_ps[i], vnat_sb[:, jc, :],
                                             p_sb, start=st_, stop=sp_,
                                             skip_group_check=True)
                            if rowz:
                                nc.tensor.matmul(
                                    zrow[i], ones_bf, p_sb,
                                    start=st_, stop=sp_,
                                    skip_group_check=True,
                                )
                            else:
                                for c in range(NC2):
                                    zo = zoff + 2 * NC2 * i + 2 * c
                                    nc.tensor.matmul(
                                        scr[:, zo:zo + 1],
                                        p_sb[:, bass.ts(c, 128)], ones_bf,
                                        start=True, stop=True,
                                    )
                        if not rowz and nrm % 2 == 1:
                            nc.vector.tensor_add(z_acc, z_acc,
                                                 scr[:, 0:8 * NC2])
                        nrm += 1
                    # diag chunks have the longest latency chains: give
                    # them double filler weight
                    o_carry += 2.0 * o_per_jc if diag else o_per_jc
                    if o_carry >= 1.0:
                        n = int(o_carry)
                        o_carry -= n
                        emit_o(n)
                if not rowz and nrm % 2 == 1:
                    # odd normal count with no restricted chunks after
                    nc.vector.tensor_add(z_acc[:, 0:4 * NC2],
                                         z_acc[:, 0:4 * NC2],
                                         scr[:, 0:4 * NC2])
                if not rowz:
                    z_f = statp.tile([128, 4 * NC2], F32, tag="zacc_f",
                                     name="z_f")
                    nc.vector.tensor_add(z_f, z_acc[:, 0:4 * NC2],
                                         z_acc[:, 4 * NC2:8 * NC2])
                for i, hh in enumerate(heads):
                    if rowz:
                        zf_sb = statp.tile([1, width], F32, tag="statf",
                                           name="zf_sb", bufs=3)
                        nc.vector.reciprocal(zf_sb, zrow[i])
                        ZR_sb = bcastp.tile([128, width], F32, tag="bcast",
                                            name="ZR_sb")
                        nc.gpsimd.partition_broadcast(ZR_sb, zf_sb)
                        nc.vector.tensor_mul(attn_slice(hh, iss), av_ps[i],
                                             ZR_sb)
                        continue
                    zr_sb = statp.tile([128, NC2, 32], F32, tag="stat4b",
                                       name="zr_sb")
                    for c in range(NC2):
                        zo = 2 * NC2 * i + 2 * c
                        nc.vector.reciprocal(
                            zr_sb[:, c, 0:1], z_f[:, zo:zo + 1])
                    zrT_ps = scr[0:32 * NC2, 64 + 128 * i:192 + 128 * i]
                    nc.tensor.transpose(zrT_ps, zr_sb, ident_sb)
                    zf_sb = statp.tile([1, width], F32, tag="statf",
                                       name="zf_sb", bufs=3)
                    for c in range(NC2):
                        nc.vector.tensor_copy(
                            zf_sb[0:1, bass.ts(c, 128)],
                            zrT_ps[32 * c:32 * c + 1, :])
                    ZR_sb = bcastp.tile([128, width], F32, tag="bcast",
                                        name="ZR_sb")
                    nc.gpsimd.partition_broadcast(ZR_sb, zf_sb)
                    nc.vector.tensor_mul(attn_slice(hh, iss), av_ps[i],
                                         ZR_sb)

            if stop_after not in ("p1", "p2"):
                for i0, width in ATILES:
                    # pieces from the previous i-tile, spread across this
                    # tile's 2 * njc j-chunk iterations
                    njc = (i0 + width) // 128
                    nc2 = width // 128
                    o_per_jc = len(o_pending) / (2.0 * (njc + nc2))
                    for hp in range(qh // 2):
                        attn_tile(hp, i0, width, o_per_jc)
                    if stop_after is None:
                        o_pending.extend(
                            (sc, ht)
                            for sc in range(i0 // 128, (i0 + width) // 128)
                            for ht in range(NHT)
                        )
                emit_o(len(o_pending))

    nc.compile()
    return nc


def make_core_inputs(hidden_states, cos, sin, norm_w, wq, wk, wv, wo,
                     s=S, h=H, qh=QH, n_cores=N_CORES):
    """Host-side sharding + layout preparation. Returns list of in_maps."""
    import ml_dtypes

    bf16 = ml_dtypes.bfloat16
    dq = qh * HD
    dkv = DKV
    x = np.asarray(hidden_states, dtype=np.float32).reshape(s, h)
    nw = np.asarray(norm_w, dtype=np.float32)
    xT = np.ascontiguousarray(x.T.astype(bf16))         # [h, s]
    cosT = np.ascontiguousarray(
        np.asarray(cos, np.float32).reshape(s, HD).T.astype(bf16))
    sinT = np.ascontiguousarray(np.asarray(sin, np.float32).reshape(s, HD).T)
    # swapped/sign-flipped sin table: rows 0:64 = +sin_half, 64:128 = -sin_half
    sin_half = sinT[0:64]
    sinTs = np.ascontiguousarray(
        np.concatenate([sinT[64:128], -sin_half], axis=0).astype(bf16))
    # fold norm_w into the projection weights
    wq_f = np.asarray(wq, np.float32) * nw[:, None]
    wk_f = np.asarray(wk, np.float32) * nw[:, None]
    wv_f = np.asarray(wv, np.float32) * nw[:, None]
    wo_f = np.asarray(wo, np.float32)

    in_maps = []
    for c in range(n_cores):
        in_maps.append({
            "xT": xT,
            "wq": np.ascontiguousarray(wq_f[:, c * dq:(c + 1) * dq].astype(bf16)),
            "wk": np.ascontiguousarray(wk_f[:, c * dkv:(c + 1) * dkv].astype(bf16)),
            "wv": np.ascontiguousarray(wv_f[:, c * dkv:(c + 1) * dkv].astype(bf16)),
            "wo": np.ascontiguousarray(wo_f[c * dq:(c + 1) * dq, :].astype(bf16)),
            "cosT": cosT,
            "sinTs": sinTs,
        })
    return in_maps


_NC_CACHE = {}


def kernel(hidden_states, cos, sin, norm_w, wq, wk, wv, wo):
    from concourse.bass_utils import run_bass_kernel_spmd

    if "nc" not in _NC_CACHE:
        _NC_CACHE["nc"] = build_bass()
    nc = _NC_CACHE["nc"]
    in_maps = make_core_inputs(hidden_states, cos, sin, norm_w, wq, wk, wv, wo)
    res = run_bass_kernel_spmd(nc, in_maps, core_ids=list(range(N_CORES)))
    partials = [m["out"] for m in res.results]
    out = np.asarray(hidden_states, np.float32).reshape(S, H).copy()
    for p in partials:
        out += np.asarray(p, dtype=np.float32)
    return out.reshape(B, S, H)


# revision 74
# speedup vs baseline: 1.4567x; 1.0007x over previous
"""Mixtral attention layer (B=1, S=2048, H=4096, NH=32, NKV=8, HD=128) on 8
Trainium2 NeuronCores, tensor-parallel over heads.

Sharding: core c owns 4 query heads + 1 KV head (column-shard of wq/wk/wv,
row-shard of wo).  Each core computes a full [S, H] partial of the o_proj
output (bf16); the host sums the 8 partials and adds the residual (the
gather of a row-parallel matmul).

Key ideas (PE matmul cost is output-free-size x cycles/row; fp32r at
free>=256 and bf16 run at 1 cycle/row):
  * Partition-dim reductions (RMSNorm sum(x^2), softmax row-sums Z) use
    FLIPPED matmuls: the data tile is the stationary operand and a ones
    vector the moving one, so each costs ~1 output column instead of the
    tile width.  Each flipped matmul is single-shot (start+stop) into a
    per-chunk-pair region of a scratch PSUM bank (concurrently-open
    accumulation groups in one PSUM bank are illegal: a start wipes the
    other open groups); a DVE add folds pairs into an SBUF accumulator.
    The [s-on-partitions] results are rotated back to [1,s] rows with a
    PE transpose (inputs spread to columns 0/32/64/96 so the transposed
    rows land on 32-aligned partition bases, which DVE reads require).
  * x stream and all weights are bf16 (walrus forbids mixing f32r with
    bf16 in one matmul, so pairs are bf16xbf16); the attention
    internals (kT/qT/vnat/probs, all accumulated in fp32 PSUM) stay
    f32r; attnT/wo (o_proj pair) and the output partials are bf16.
  * Phase 1 fuses norm stats into the projection stream (x read once):
    per H-chunk, 6 accumulating projection matmuls + ACT square + 4
    flipped stats matmuls (lagged 4 chunks so tile starts are pure
    projection work).  Weight DMAs are chunked and interleaved with
    tile-0 x chunks so the PE starts ~4us in; wo streams during tiles
    1-3 into its own SBUF slot; v is transposed to natural [j, d]
    layout mid-next-tile.  norm_w folds into the weights on the host,
    the RMSNorm 1/rms into the RoPE cos/sin tables; q^T/k^T go straight
    to persistent SBUF (no DRAM spill).
  * Attention (causal flash-style, per head-pair per 512-wide i-tile):
    scores^T = kT.T @ qT chunkwise, exp on ACT, causal mask via GPSIMD
    affine_select on diagonal blocks, unnormalized AV accumulating in
    PSUM, Z via flipped matmuls; 1/Z applied at AV evacuation.  The
    previous i-tile's o_proj pieces are emitted one at a time between
    j-chunks so the scheduler can fill the exp->mask->AV latency chains
    with o_proj matmuls.
"""

import math

import numpy as np

import concourse.bass as bass
import concourse.tile as tile
from concourse import bacc, mybir
from concourse.masks import make_identity

F32 = mybir.dt.float32
F32R = mybir.dt.float32r
BF16 = mybir.dt.bfloat16

# Full problem dims
B, S, H, NH, NKV, HD = 1, 2048, 4096, 32, 8, 128
EPS = 1e-5
N_CORES = 8
QH = NH // N_CORES          # query heads per core = 4
DQ = QH * HD                # q columns per core = 512
DKV = (NKV // N_CORES) * HD  # kv columns per core = 128


def build_bass(s=S, h=H, qh=QH, stop_after=None):
    """Build the single-core Bass module (same NEFF on all 8 cores)."""
    ST = 512 if s >= 512 else s       # s-tile width (proj + attention i-tiles)
    NST = s // ST                     # number of s-tiles
    HC = h // 128                     # H contraction chunks
    NJ = s // 128                     # j chunks (keys)
    NHT = h // 512 if h >= 512 else 1  # h tiles for o_proj output
    HT = min(512, h)
    WGRP = 4                          # h-chunks per weight DMA group
    dq = qh * HD
    scale = 1.0 / math.sqrt(HD)

    nc = bacc.Bacc(None, target_bir_lowering=False)

    xT = nc.dram_tensor("xT", [h, s], BF16, kind="ExternalInput")
    wq = nc.dram_tensor("wq", [h, dq], BF16, kind="ExternalInput")
    wk = nc.dram_tensor("wk", [h, DKV], BF16, kind="ExternalInput")
    wv = nc.dram_tensor("wv", [h, DKV], BF16, kind="ExternalInput")
    wo = nc.dram_tensor("wo", [dq, h], BF16, kind="ExternalInput")
    cosT = nc.dram_tensor("cosT", [HD, s], BF16, kind="ExternalInput")
    sinTs = nc.dram_tensor("sinTs", [HD, s], BF16, kind="ExternalInput")
    out = nc.dram_tensor("out", [s, h], BF16, kind="ExternalOutput")

    xT_t = xT.rearrange("(ho hi) s -> hi ho s", hi=128)
    wq_t = wq.rearrange("(ho hi) d -> hi ho d", hi=128)
    wk_t = wk.rearrange("(ho hi) d -> hi ho d", hi=128)
    wv_t = wv.rearrange("(ho hi) d -> hi ho d", hi=128)
    wo_t = wo.rearrange("(do di) h -> di do h", di=128)

    with tile.TileContext(nc) as tc:
        with (
            tc.tile_pool(name="persist", bufs=1) as persist,
            tc.tile_pool(name="xin", bufs=4) as xin,
            tc.tile_pool(name="x2b", bufs=9) as x2b,
            tc.tile_pool(name="rope", bufs=3) as ropep,
            tc.tile_pool(name="statp", bufs=6) as statp,
            tc.tile_pool(name="tabp", bufs=3) as tabp,
            tc.tile_pool(name="bcastp", bufs=3) as bcastp,
            tc.tile_pool(name="probs", bufs=6) as probs,
            tc.tile_pool(name="outp", bufs=6) as outp,
            tc.tile_pool(name="acc_ps", bufs=8, space="PSUM") as acc_ps,
        ):
            # ---- persistent SBUF tensors ----
            # Slot reuse chains (same tag, sequential lifetimes):
            #   wq (8MB) -> wo (8MB)         tag "bigw"
            #   wk (2MB) -> attnT heads 0-1  tag "wk"
            #   wv (2MB) -> attnT heads 2-3  tag "wv"
            #   cos (1MB) -> v natural (1MB) tag "cosvnat"
            wq_sb = persist.tile([128, HC, dq], BF16, tag="bigw")
            wk_sb = persist.tile([128, HC, DKV], BF16, tag="wk")
            wv_sb = persist.tile([128, HC, DKV], BF16, tag="wv")
            cos_sb = persist.tile([128, s], BF16, tag="cos")
            sin_sb = persist.tile([128, s], BF16, tag="sin")
            ones_f = persist.tile([128, 1], F32, tag="ones_f")
            ones_sb = persist.tile([128, 2], F32R, tag="ones")
            ones_bf = persist.tile([128, 1], BF16, tag="ones_bf")
            eps_sb = persist.tile([128, 1], F32, tag="eps")
            ident_sb = persist.tile([128, 128], F32, tag="ident")
            kT_sb = persist.tile([128, s], BF16, tag="kT")
            vT_sb = persist.tile([128, s], F32, tag="vT")
            qT_sb = persist.tile([128, qh, s], BF16, tag="qT")
            wo_sb = persist.tile([128, qh, h], BF16, tag="wo")
            vnat_sb = persist.tile([128, NJ, 128], BF16, tag="vnat")

            nc.vector.memset(ones_f, 1.0)
            nc.scalar.copy(ones_sb[:, 0:1], ones_f)
            nc.scalar.copy(ones_sb[:, 1:2], ones_f)
            nc.scalar.copy(ones_bf, ones_f)
            nc.vector.memset(eps_sb, EPS)
            make_identity(nc, ident_sb)

            # ---- phase 1: fused norm stats + q/k/v projections off a single
            # fp32r x^T stream; weight DMAs interleaved with tile-0 x chunks
            def pass_b(st):
                ss = bass.ts(st, ST)
                q_ps = [acc_ps.tile([128, ST], F32, tag="acc", name=f"q_ps{m}")
                        for m in range(qh)]
                k_ps = acc_ps.tile([128, ST], F32, tag="acc", name="k_ps")
                v_ps = acc_ps.tile([128, ST], F32, tag="acc", name="v_ps")
                # sum(x^2) accumulates in SBUF: each chunk's flipped
                # matmuls are single-shot (start+stop) into a transient PSUM
                # tile -- concurrently-open accumulation groups in one PSUM
                # bank are illegal (a start wipes the other open groups)
                # one scratch PSUM bank per s-tile: cols 0:8 hold the
                # per-chunk-pair stats groups, 16:144 the r transpose
                scr = acc_ps.tile([128, 144], F32, tag="acc", name="p1scr")
                sq_acc = statp.tile([128, 8], F32, tag="sqacc",
                                    name="sq_acc")
                nc.vector.memset(sq_acc, 0.0)
                # weight DMA groups for tile 0: small groups first so the
                # PE can start early, then 4-chunk groups
                wgroups = [(0, 1), (1, 1), (2, 2)] + [
                    (g, WGRP) for g in range(4, HC, WGRP)]

                def stats(hc):
                    # flipped stats: sum over h of x^2 lands on the s
                    # partitions; costs ~1 output column per matmul.  Each
                    # matmul is single-shot (concurrently-open accumulation
                    # groups in one PSUM bank are illegal); a chunk pair
                    # lands in scratch cols 0:8, then one DVE add folds it
                    # into the SBUF accumulator.
                    off = 4 * (hc % 2)
                    for c in range(4):
                        nc.tensor.matmul(
                            scr[:, off + c:off + c + 1],
                            x2s[hc][:, bass.ts(c, 128)],
                            ones_bf, start=True, stop=True,
                        )
                    if hc % 2 == 1:
                        nc.vector.tensor_add(sq_acc, sq_acc, scr[:, 0:8])
                    x2s[hc] = None

                x2s = {}
                for hc in range(HC):
                    x_sb = xin.tile([128, ST], BF16)
                    nc.sync.dma_start(out=x_sb, in_=xT_t[:, hc, ss])
                    if st == 0 and wgroups and wgroups[0][0] == hc:
                        g0, gn = wgroups.pop(0)
                        nc.sync.dma_start(out=wq_sb[:, g0:g0 + gn, :],
                                          in_=wq_t[:, g0:g0 + gn, :])
                        nc.sync.dma_start(out=wk_sb[:, g0:g0 + gn, :],
                                          in_=wk_t[:, g0:g0 + gn, :])
                        nc.sync.dma_start(out=wv_sb[:, g0:g0 + gn, :],
                                          in_=wv_t[:, g0:g0 + gn, :])
                    if st > 0 and hc in (6, 14, 22):
                        # o_proj weights trickle in during tiles 1-3
                        ht = 3 * (st - 1) + (hc - 6) // 8
                        if ht < NHT:
                            nc.sync.dma_start(
                                out=wo_sb[:, :, bass.ts(ht, HT)],
                                in_=wo_t[:, :, bass.ts(ht, HT)],
                            )
                    x2_sb = x2b.tile([128, ST], BF16)
                    nc.scalar.square(x2_sb, x_sb)
                    x2s[hc] = x2_sb
                    st_, sp_ = (hc == 0), (hc == HC - 1)
                    for m in range(qh):
                        nc.tensor.matmul(
                            q_ps[m], wq_sb[:, hc, bass.ts(m, 128)], x_sb,
                            start=st_, stop=sp_,
                        )
                    nc.tensor.matmul(k_ps, wk_sb[:, hc, :], x_sb,
                                     start=st_, stop=sp_)
                    nc.tensor.matmul(v_ps, wv_sb[:, hc, :], x_sb,
                                     start=st_, stop=sp_)
                    # stats lag the stream so tile starts are pure
                    # projection work
                    if hc >= 8:
                        stats(hc - 8)
                    # previous s-tile's v -> natural [j, d] transposes,
                    # placed mid-tile where PSUM banks have slack
                    if st > 0 and hc in (16, 18, 20, 22):
                        jc = (st - 1) * (ST // 128) + (hc - 16) // 2
                        vt_ps = acc_ps.tile([128, 128], F32, tag="acc")
                        nc.tensor.transpose(
                            vt_ps, vT_sb[:, bass.ts(jc, 128)], ident_sb)
                        nc.scalar.copy(vnat_sb[:, jc, :], vt_ps)
                if st == 0:
                    # rope tables: after tile-0's weights, before the first
                    # evacuation needs them
                    nc.sync.dma_start(out=cos_sb, in_=cosT[:, :])
                    nc.sync.dma_start(out=sin_sb, in_=sinTs[:, :])
                for hcl in range(HC - 8, HC):
                    stats(hcl)
                # r = 1/sqrt(mean + eps) in [s-part, 4] layout, rotated back
                # to a [1, ST] row for the table broadcast
                sq_f = statp.tile([128, 4], F32, tag="stat4f",
                                  name="sq_f")
                nc.vector.tensor_add(sq_f, sq_acc[:, 0:4], sq_acc[:, 4:8])
                sd_sb = statp.tile([128, 4], F32, tag="stat4",
                                   name="sd_sb")
                nc.scalar.activation(
                    sd_sb, sq_f, mybir.ActivationFunctionType.Sqrt,
                    bias=eps_sb, scale=1.0 / h,
                )
                # reciprocals spread to columns 0/32/64/96 so the
                # transpose lands them on 32-aligned partitions (DVE reads
                # require 32-aligned partition bases)
                rr_sb = statp.tile([128, 4, 32], F32, tag="stat4b",
                                   name="rr_sb")
                for c in range(4):
                    nc.vector.reciprocal(rr_sb[:, c, 0:1], sd_sb[:, c:c + 1])
                rT_ps = scr[:, 16:144]
                nc.tensor.transpose(rT_ps, rr_sb, ident_sb)
                rf_sb = statp.tile([1, ST], BF16, tag="statfb",
                                   name="rf_sb", bufs=3)
                for c in range(4):
                    nc.vector.tensor_copy(
                        rf_sb[0:1, bass.ts(c, 128)],
                        rT_ps[32 * c:32 * c + 1, :])
                R_t = tabp.tile([128, ST], BF16, tag="R", name="R_t")
                nc.gpsimd.partition_broadcast(R_t, rf_sb)
                cp_t = tabp.tile([128, ST], BF16, tag="cp", name="cp_t")
                nc.vector.tensor_mul(cp_t, cos_sb[:, ss], R_t)
                sp_t = tabp.tile([128, ST], BF16, tag="sp", name="sp_t")
                nc.vector.tensor_mul(sp_t, sin_sb[:, ss], R_t)

                # evacuation: fast ACT copy frees the PSUM bank, then
                # norm+RoPE happens SBUF-side on DVE (in place; the u-halves
                # read the raw values before the cos-multiply overwrites)
                # all PSUM->SBUF copies first, alternating ACT/DVE, so
                # the banks free ~2x faster for the next tile; norm+RoPE
                # then happens SBUF-side on DVE (in place; the u-halves
                # read the raw values before the cos-multiply overwrites)
                evacs = [(k_ps, kT_sb[:, ss])]
                evacs += [(q_ps[m], qT_sb[:, m, ss]) for m in range(qh)]
                for idx, (src_ps, dst) in enumerate(evacs):
                    if idx % 2 == 0:
                        nc.scalar.copy(dst, src_ps)
                    else:
                        nc.vector.tensor_copy(dst, src_ps)
                nc.scalar.copy(vT_sb[:, ss], v_ps)

                def rope_rot(dst):
                    u_sb = ropep.tile([128, ST], BF16, tag="u",
                                      name="u_sb", bufs=3)
                    nc.vector.tensor_mul(
                        u_sb[0:64, :], dst[64:128, :], sp_t[64:128, :])
                    nc.vector.tensor_mul(
                        u_sb[64:128, :], dst[0:64, :], sp_t[0:64, :])
                    nc.vector.tensor_mul(dst, dst, cp_t)
                    nc.vector.tensor_add(dst, dst, u_sb)

                for _, dst in evacs:
                    rope_rot(dst)
                nc.vector.tensor_mul(vT_sb[:, ss], vT_sb[:, ss], R_t)

            for st in range(NST):
                pass_b(st)

            # ---- phase 2: last s-tile's v transposes + the last wo
            # chunks (the rest streamed during tiles 1-3) ----
            if stop_after != "p1":
                for ht in range(min(3 * (NST - 1), NHT), NHT):
                    nc.sync.dma_start(
                        out=wo_sb[:, :, bass.ts(ht, HT)],
                        in_=wo_t[:, :, bass.ts(ht, HT)],
                    )
            for jc in range((NST - 1) * (ST // 128),
                            NJ if stop_after != "p1" else 0):
                vt_ps = acc_ps.tile([128, 128], F32, tag="acc")
                nc.tensor.transpose(vt_ps, vT_sb[:, bass.ts(jc, 128)],
                                    ident_sb)
                nc.scalar.copy(vnat_sb[:, jc, :], vt_ps)

            # attn^T reuses the wk/wv slots (heads 0-1 / 2-3)
            attnT_h = [
                persist.tile([128, 2, s], BF16, tag="wk", name="attnT01"),
                persist.tile([128, 2, s], BF16, tag="wv", name="attnT23"),
            ]

            def attn_slice(m, sl):
                return attnT_h[m // 2][:, m % 2, sl]

            # ---- phase 3 + 4 interleaved: attention per i-tile (both head
            # pairs); the previous i-tile's o_proj pieces are emitted one at
            # a time between j-chunks so they are available as PE filler
            # during the exp->mask->AV latency chains
            o_pending = []

            def o_proj_piece(sc, ht):
                scs = bass.ts(sc, 128)
                o_ps = acc_ps.tile([128, HT], F32, tag="acc")
                for m in range(qh):
                    nc.tensor.matmul(
                        o_ps, attn_slice(m, scs),
                        wo_sb[:, m, bass.ts(ht, HT)],
                        start=(m == 0), stop=(m == qh - 1),
                    )
                o_sb = outp.tile([128, HT], BF16)
                if (sc + ht) % 2 == 0:
                    nc.scalar.copy(o_sb, o_ps)
                else:
                    nc.vector.tensor_copy(o_sb, o_ps)
                nc.sync.dma_start(
                    out=out[scs, bass.ts(ht, HT)], in_=o_sb
                )

            def emit_o(n):
                for _ in range(n):
                    if o_pending:
                        o_proj_piece(*o_pending.pop(0))

            # attention i-tiles: narrow at the start (shrinks the
            # filler-less warmup) and at the end (shrinks the un-overlapped
            # o_proj tail), wide in the middle; narrower diag tiles also
            # skip more of the causal upper triangle
            ATILES = [(0, 256), (256, 256), (512, 512), (1024, 512),
                      (1536, 512)]

            def attn_tile(hp, i0, width, o_per_jc):
                NC2 = width // 128
                heads = (2 * hp, 2 * hp + 1)
                iss = slice(i0, i0 + width)
                av_ps = [acc_ps.tile([128, width], F32, tag="acc",
                                     name=f"av_ps{i}") for i in range(2)]
                njc = (i0 + width) // 128
                # small warmup tiles: Z via the direct ones-matmul into a
                # [1, w] row (cheap at small njc*w, and the row layout needs
                # no transpose/copies -- a much shorter evac chain).  Large
                # tiles use flipped single-shot matmuls + SBUF accumulation.
                rowz = njc <= 4
                if rowz:
                    zrow = [acc_ps.tile([1, width], F32, tag="acc",
                                        name=f"zrow{i}") for i in range(2)]
                else:
                    scr = acc_ps.tile([128, 320], F32, tag="acc",
                                      name="a_scr")
                    z_acc = statp.tile([128, 8 * NC2], F32, tag="zacc",
                                       name="z_acc")
                    nc.vector.memset(z_acc, 0.0)
                o_carry = 0.0
                nrm = 0      # normal-chunk counter for pair batching
                for jc in range(njc):
                    st_, sp_ = (jc == 0), (jc == njc - 1)
                    rel = jc * 128 - i0
                    diag = rel + 128 > 0
                    # diag chunks only need query columns >= the diagonal;
                    # restrict on wide tiles (bf16 scores run 1 cyc/row at
                    # any width; the f32r AV at 128 wide costs the same 4x)
                    restr = diag and rel > 0
                    if restr:
                        if not rowz and nrm % 2 == 1:
                            # fold the unpaired normal chunk's parity-0
                            # region before restricted chunks reuse it
                            nc.vector.tensor_add(z_acc[:, 0:4 * NC2],
                                                 z_acc[:, 0:4 * NC2],
                                                 scr[:, 0:4 * NC2])
                            nrm += 1
                        nskip = rel // 128
                        for i in range(2):
                            s_ps = acc_ps.tile([128, width], F32, tag="acc",
                                               name=f"s_ps{i}")
                            nc.tensor.matmul(
                                s_ps[:, rel:width],
                                kT_sb[:, bass.ts(jc, 128)],
                                qT_sb[:, heads[i], i0 + rel:i0 + width],
                                start=True, stop=True,
                            )
                            p_sb = probs.tile([128, width], BF16, tag="p",
                                              name=f"p_sb{i}", bufs=10)
                            nc.scalar.activation(
                                p_sb[:, rel:width], s_ps[:, rel:width],
                                mybir.ActivationFunctionType.Exp,
                                scale=scale,
                            )
                            nc.gpsimd.affine_select(
                                out=p_sb[:, rel:rel + 128],
                                in_=p_sb[:, rel:rel + 128],
                                pattern=[[1, 128]],
                                compare_op=mybir.AluOpType.is_ge,
                                fill=0.0, base=0, channel_multiplier=-1,
                            )
                            nc.tensor.matmul(
                                av_ps[i][:, rel:width], vnat_sb[:, jc, :],
                                p_sb[:, rel:width],
                                start=False, stop=sp_,
                                skip_group_check=True,
                            )
                            if rowz:
                                nc.tensor.matmul(
                                    zrow[i][:, rel:width], ones_bf,
                                    p_sb[:, rel:width],
                                    start=False, stop=sp_,
                                    skip_group_check=True,
                                )
                            else:
                                for c in range(nskip, NC2):
                                    zo = 2 * NC2 * i + 2 * c
                                    nc.tensor.matmul(
                                        scr[:, zo:zo + 1],
                                        p_sb[:, bass.ts(c, 128)], ones_bf,
                                        start=True, stop=True,
                                    )
                        if not rowz:
                            # per-chunk fold of the freshly-written slots
                            for i in range(2):
                                zo = 2 * NC2 * i + 2 * nskip
                                hi = 2 * NC2 * (i + 1)
                                nc.vector.tensor_add(
                                    z_acc[:, zo:hi], z_acc[:, zo:hi],
                                    scr[:, zo:hi])
                    else:
                        zoff = 4 * NC2 * (nrm % 2)
                        for i in range(2):
                            s_ps = acc_ps.tile([128, width], F32, tag="acc",
                                               name=f"s_ps{i}")
                            nc.tensor.matmul(
                                s_ps, kT_sb[:, bass.ts(jc, 128)],
                                qT_sb[:, heads[i], iss],
                                start=True, stop=True,
                            )
                            p_sb = probs.tile([128, width], BF16, tag="p",
                                              name=f"p_sb{i}", bufs=10)
                            nc.scalar.activation(
                                p_sb, s_ps,
                                mybir.ActivationFunctionType.Exp,
                                scale=scale,
                            )
                            if diag:
                                # rel == 0 here: only the first 128-col
                                # block is triangular; later columns are
                                # always kept, so mask just that block
                                nc.gpsimd.affine_select(
                                    out=p_sb[:, 0:128], in_=p_sb[:, 0:128],
                                    pattern=[[1, 128]],
                                    compare_op=mybir.AluOpType.is_ge,
                                    fill=0.0,
                                    base=0,
                                    channel_multiplier=-1,
                                )
                            nc.tensor.matmul(av_ps[i], vnat_sb[:, jc, :],
                                             p_sb, start=st_, stop=sp_,
                                             skip_group_check=True)
                            if rowz:
                                nc.tensor.matmul(
                                    zrow[i], ones_bf, p_sb,
                                    start=st_, stop=sp_,
                                    skip_group_check=True,
                                )
                            else:
                                for c in range(NC2):
                                    zo = zoff + 2 * NC2 * i + 2 * c
                                    nc.tensor.matmul(
                                        scr[:, zo:zo + 1],
                                        p_sb[:, bass.ts(c, 128)], ones_bf,
                                        start=True, stop=True,
                                    )
                        if not rowz and nrm % 2 == 1:
                            nc.vector.tensor_add(z_acc, z_acc,
                                                 scr[:, 0:8 * NC2])
                        nrm += 1
                    # diag chunks have the longest latency chains: give
                    # them double filler weight
                    o_carry += 2.0 * o_per_jc if diag else o_per_jc
                    if o_carry >= 1.0:
                        n = int(o_carry)
                        o_carry -= n
                        emit_o(n)
                if not rowz and nrm % 2 == 1:
                    # odd normal count with no restricted chunks after
                    nc.vector.tensor_add(z_acc[:, 0:4 * NC2],
                                         z_acc[:, 0:4 * NC2],
                                         scr[:, 0:4 * NC2])
                if not rowz:
                    z_f = statp.tile([128, 4 * NC2], F32, tag="zacc_f",
                                     name="z_f")
                    nc.vector.tensor_add(z_f, z_acc[:, 0:4 * NC2],
                                         z_acc[:, 4 * NC2:8 * NC2])
                for i, hh in enumerate(heads):
                    if rowz:
                        zf_sb = statp.tile([1, width], F32, tag="statf",
                                           name="zf_sb", bufs=3)
                        nc.vector.reciprocal(zf_sb, zrow[i])
                        ZR_sb = bcastp.tile([128, width], F32, tag="bcast",
                                            name="ZR_sb")
                        nc.gpsimd.partition_broadcast(ZR_sb, zf_sb)
                        nc.vector.tensor_mul(attn_slice(hh, iss), av_ps[i],
                                             ZR_sb)
                        continue
                    zr_sb = statp.tile([128, NC2, 32], F32, tag="stat4b",
                                       name="zr_sb")
                    for c in range(NC2):
                        zo = 2 * NC2 * i + 2 * c
                        nc.vector.reciprocal(
                            zr_sb[:, c, 0:1], z_f[:, zo:zo + 1])
                    zrT_ps = scr[0:32 * NC2, 64 + 128 * i:192 + 128 * i]
                    nc.tensor.transpose(zrT_ps, zr_sb, ident_sb)
                    zf_sb = statp.tile([1, width], F32, tag="statf",
                                       name="zf_sb", bufs=3)
                    for c in range(NC2):
                        nc.vector.tensor_copy(
                            zf_sb[0:1, bass.ts(c, 128)],
                            zrT_ps[32 * c:32 * c + 1, :])
                    ZR_sb = bcastp.tile([128, width], F32, tag="bcast",
                                        name="ZR_sb")
                    nc.gpsimd.partition_broadcast(ZR_sb, zf_sb)
                    nc.vector.tensor_mul(attn_slice(hh, iss), av_ps[i],
                                         ZR_sb)

            if stop_after not in ("p1", "p2"):
                for i0, width in ATILES:
                    # pieces from the previous i-tile, spread across this
                    # tile's 2 * njc j-chunk iterations
                    njc = (i0 + width) // 128
                    nc2 = width // 128
                    o_per_jc = len(o_pending) / (2.0 * (njc + nc2))
                    for hp in range(qh // 2):
                        attn_tile(hp, i0, width, o_per_jc)
                    if stop_after is None:
                        o_pending.extend(
                            (sc, ht)
                            for sc in range(i0 // 128, (i0 + width) // 128)
                            for ht in range(NHT)
                        )
                emit_o(len(o_pending))

    nc.compile()
    return nc


def make_core_inputs(hidden_states, cos, sin, norm_w, wq, wk, wv, wo,
                     s=S, h=H, qh=QH, n_cores=N_CORES):
    """Host-side sharding + layout preparation. Returns list of in_maps."""
    import ml_dtypes

    bf16 = ml_dtypes.bfloat16
    dq = qh * HD
    dkv = DKV
    x = np.asarray(hidden_states, dtype=np.float32).reshape(s, h)
    nw = np.asarray(norm_w, dtype=np.float32)
    xT = np.ascontiguousarray(x.T.astype(bf16))         # [h, s]
    cosT = np.ascontiguousarray(
        np.asarray(cos, np.float32).reshape(s, HD).T.astype(bf16))
    sinT = np.ascontiguousarray(np.asarray(sin, np.float32).reshape(s, HD).T)
    # swapped/sign-flipped sin table: rows 0:64 = +sin_half, 64:128 = -sin_half
    sin_half = sinT[0:64]
    sinTs = np.ascontiguousarray(
        np.concatenate([sinT[64:128], -sin_half], axis=0).astype(bf16))
    # fold norm_w into the projection weights
    wq_f = np.asarray(wq, np.float32) * nw[:, None]
    wk_f = np.asarray(wk, np.float32) * nw[:, None]
    wv_f = np.asarray(wv, np.float32) * nw[:, None]
    wo_f = np.asarray(wo, np.float32)

    in_maps = []
    for c in range(n_cores):
        in_maps.append({
            "xT": xT,
            "wq": np.ascontiguousarray(wq_f[:, c * dq:(c + 1) * dq].astype(bf16)),
            "wk": np.ascontiguousarray(wk_f[:, c * dkv:(c + 1) * dkv].astype(bf16)),
            "wv": np.ascontiguousarray(wv_f[:, c * dkv:(c + 1) * dkv].astype(bf16)),
            "wo": np.ascontiguousarray(wo_f[c * dq:(c + 1) * dq, :].astype(bf16)),
            "cosT": cosT,
            "sinTs": sinTs,
        })
    return in_maps


_NC_CACHE = {}


def kernel(hidden_states, cos, sin, norm_w, wq, wk, wv, wo):
    from concourse.bass_utils import run_bass_kernel_spmd

    if "nc" not in _NC_CACHE:
        _NC_CACHE["nc"] = build_bass()
    nc = _NC_CACHE["nc"]
    in_maps = make_core_inputs(hidden_states, cos, sin, norm_w, wq, wk, wv, wo)
    res = run_bass_kernel_spmd(nc, in_maps, core_ids=list(range(N_CORES)))
    partials = [m["out"] for m in res.results]
    out = np.asarray(hidden_states, np.float32).reshape(S, H).copy()
    for p in partials:
        out += np.asarray(p, dtype=np.float32)
    return out.reshape(B, S, H)


# revision 75
# speedup vs baseline: 1.4626x; 1.0040x over previous
"""Mixtral attention layer (B=1, S=2048, H=4096, NH=32, NKV=8, HD=128) on 8
Trainium2 NeuronCores, tensor-parallel over heads.

Sharding: core c owns 4 query heads + 1 KV head (column-shard of wq/wk/wv,
row-shard of wo).  Each core computes a full [S, H] partial of the o_proj
output (bf16); the host sums the 8 partials and adds the residual (the
gather of a row-parallel matmul).

Key ideas (PE matmul cost is output-free-size x cycles/row; fp32r at
free>=256 and bf16 run at 1 cycle/row):
  * Partition-dim reductions (RMSNorm sum(x^2), softmax row-sums Z) use
    FLIPPED matmuls: the data tile is the stationary operand and a ones
    vector the moving one, so each costs ~1 output column instead of the
    tile width.  Each flipped matmul is single-shot (start+stop) into a
    per-chunk-pair region of a scratch PSUM bank (concurrently-open
    accumulation groups in one PSUM bank are illegal: a start wipes the
    other open groups); a DVE add folds pairs into an SBUF accumulator.
    The [s-on-partitions] results are rotated back to [1,s] rows with a
    PE transpose (inputs spread to columns 0/32/64/96 so the transposed
    rows land on 32-aligned partition bases, which DVE reads require).
  * x stream and all weights are bf16 (walrus forbids mixing f32r with
    bf16 in one matmul, so pairs are bf16xbf16); the attention
    internals (kT/qT/vnat/probs, all accumulated in fp32 PSUM) stay
    f32r; attnT/wo (o_proj pair) and the output partials are bf16.
  * Phase 1 fuses norm stats into the projection stream (x read once):
    per H-chunk, 6 accumulating projection matmuls + ACT square + 4
    flipped stats matmuls (lagged 4 chunks so tile starts are pure
    projection work).  Weight DMAs are chunked and interleaved with
    tile-0 x chunks so the PE starts ~4us in; wo streams during tiles
    1-3 into its own SBUF slot; v is transposed to natural [j, d]
    layout mid-next-tile.  norm_w folds into the weights on the host,
    the RMSNorm 1/rms into the RoPE cos/sin tables; q^T/k^T go straight
    to persistent SBUF (no DRAM spill).
  * Attention (causal flash-style, per head-pair per 512-wide i-tile):
    scores^T = kT.T @ qT chunkwise, exp on ACT, causal mask via GPSIMD
    affine_select on diagonal blocks, unnormalized AV accumulating in
    PSUM, Z via flipped matmuls; 1/Z applied at AV evacuation.  The
    previous i-tile's o_proj pieces are emitted one at a time between
    j-chunks so the scheduler can fill the exp->mask->AV latency chains
    with o_proj matmuls.
"""

import math

import numpy as np

import concourse.bass as bass
import concourse.tile as tile
from concourse import bacc, mybir
from concourse.masks import make_identity

F32 = mybir.dt.float32
F32R = mybir.dt.float32r
BF16 = mybir.dt.bfloat16

# Full problem dims
B, S, H, NH, NKV, HD = 1, 2048, 4096, 32, 8, 128
EPS = 1e-5
N_CORES = 8
QH = NH // N_CORES          # query heads per core = 4
DQ = QH * HD                # q columns per core = 512
DKV = (NKV // N_CORES) * HD  # kv columns per core = 128


def build_bass(s=S, h=H, qh=QH, stop_after=None):
    """Build the single-core Bass module (same NEFF on all 8 cores)."""
    ST = 512 if s >= 512 else s       # s-tile width (proj + attention i-tiles)
    NST = s // ST                     # number of s-tiles
    HC = h // 128                     # H contraction chunks
    NJ = s // 128                     # j chunks (keys)
    NHT = h // 512 if h >= 512 else 1  # h tiles for o_proj output
    HT = min(512, h)
    WGRP = 4                          # h-chunks per weight DMA group
    dq = qh * HD
    scale = 1.0 / math.sqrt(HD)

    nc = bacc.Bacc(None, target_bir_lowering=False)

    xT = nc.dram_tensor("xT", [h, s], BF16, kind="ExternalInput")
    wq = nc.dram_tensor("wq", [h, dq], BF16, kind="ExternalInput")
    wk = nc.dram_tensor("wk", [h, DKV], BF16, kind="ExternalInput")
    wv = nc.dram_tensor("wv", [h, DKV], BF16, kind="ExternalInput")
    wo = nc.dram_tensor("wo", [dq, h], BF16, kind="ExternalInput")
    cosT = nc.dram_tensor("cosT", [HD, s], BF16, kind="ExternalInput")
    sinTs = nc.dram_tensor("sinTs", [HD, s], BF16, kind="ExternalInput")
    out = nc.dram_tensor("out", [s, h], BF16, kind="ExternalOutput")

    xT_t = xT.rearrange("(ho hi) s -> hi ho s", hi=128)
    wq_t = wq.rearrange("(ho hi) d -> hi ho d", hi=128)
    wk_t = wk.rearrange("(ho hi) d -> hi ho d", hi=128)
    wv_t = wv.rearrange("(ho hi) d -> hi ho d", hi=128)
    wo_t = wo.rearrange("(do di) h -> di do h", di=128)

    with tile.TileContext(nc) as tc:
        with (
            tc.tile_pool(name="persist", bufs=1) as persist,
            tc.tile_pool(name="xin", bufs=5) as xin,
            tc.tile_pool(name="x2b", bufs=9) as x2b,
            tc.tile_pool(name="rope", bufs=3) as ropep,
            tc.tile_pool(name="statp", bufs=6) as statp,
            tc.tile_pool(name="tabp", bufs=3) as tabp,
            tc.tile_pool(name="bcastp", bufs=3) as bcastp,
            tc.tile_pool(name="probs", bufs=6) as probs,
            tc.tile_pool(name="outp", bufs=6) as outp,
            tc.tile_pool(name="acc_ps", bufs=8, space="PSUM") as acc_ps,
        ):
            # ---- persistent SBUF tensors ----
            # Slot reuse chains (same tag, sequential lifetimes):
            #   wq (8MB) -> wo (8MB)         tag "bigw"
            #   wk (2MB) -> attnT heads 0-1  tag "wk"
            #   wv (2MB) -> attnT heads 2-3  tag "wv"
            #   cos (1MB) -> v natural (1MB) tag "cosvnat"
            wq_sb = persist.tile([128, HC, dq], BF16, tag="bigw")
            wk_sb = persist.tile([128, HC, DKV], BF16, tag="wk")
            wv_sb = persist.tile([128, HC, DKV], BF16, tag="wv")
            cos_sb = persist.tile([128, s], BF16, tag="cos")
            sin_sb = persist.tile([128, s], BF16, tag="sin")
            ones_f = persist.tile([128, 1], F32, tag="ones_f")
            ones_sb = persist.tile([128, 2], F32R, tag="ones")
            ones_bf = persist.tile([128, 1], BF16, tag="ones_bf")
            eps_sb = persist.tile([128, 1], F32, tag="eps")
            ident_sb = persist.tile([128, 128], F32, tag="ident")
            kT_sb = persist.tile([128, s], BF16, tag="kT")
            vT_sb = persist.tile([128, s], F32, tag="vT")
            qT_sb = persist.tile([128, qh, s], BF16, tag="qT")
            wo_sb = persist.tile([128, qh, h], BF16, tag="wo")
            vnat_sb = persist.tile([128, NJ, 128], BF16, tag="vnat")

            nc.vector.memset(ones_f, 1.0)
            nc.scalar.copy(ones_sb[:, 0:1], ones_f)
            nc.scalar.copy(ones_sb[:, 1:2], ones_f)
            nc.scalar.copy(ones_bf, ones_f)
            nc.vector.memset(eps_sb, EPS)
            make_identity(nc, ident_sb)

            # ---- phase 1: fused norm stats + q/k/v projections off a single
            # fp32r x^T stream; weight DMAs interleaved with tile-0 x chunks
            def pass_b(st):
                ss = bass.ts(st, ST)
                q_ps = [acc_ps.tile([128, ST], F32, tag="acc", name=f"q_ps{m}")
                        for m in range(qh)]
                k_ps = acc_ps.tile([128, ST], F32, tag="acc", name="k_ps")
                v_ps = acc_ps.tile([128, ST], F32, tag="acc", name="v_ps")
                # sum(x^2) accumulates in SBUF: each chunk's flipped
                # matmuls are single-shot (start+stop) into a transient PSUM
                # tile -- concurrently-open accumulation groups in one PSUM
                # bank are illegal (a start wipes the other open groups)
                # one scratch PSUM bank per s-tile: cols 0:8 hold the
                # per-chunk-pair stats groups, 16:144 the r transpose
                scr = acc_ps.tile([128, 144], F32, tag="acc", name="p1scr")
                sq_acc = statp.tile([128, 8], F32, tag="sqacc",
                                    name="sq_acc")
                nc.vector.memset(sq_acc, 0.0)
                # weight DMA groups for tile 0: small groups first so the
                # PE can start early, then 4-chunk groups
                wgroups = [(0, 1), (1, 1), (2, 2)] + [
                    (g, WGRP) for g in range(4, HC, WGRP)]

                def stats(hc):
                    # flipped stats: sum over h of x^2 lands on the s
                    # partitions; costs ~1 output column per matmul.  Each
                    # matmul is single-shot (concurrently-open accumulation
                    # groups in one PSUM bank are illegal); a chunk pair
                    # lands in scratch cols 0:8, then one DVE add folds it
                    # into the SBUF accumulator.
                    off = 4 * (hc % 2)
                    for c in range(4):
                        nc.tensor.matmul(
                            scr[:, off + c:off + c + 1],
                            x2s[hc][:, bass.ts(c, 128)],
                            ones_bf, start=True, stop=True,
                        )
                    if hc % 2 == 1:
                        nc.vector.tensor_add(sq_acc, sq_acc, scr[:, 0:8])
                    x2s[hc] = None

                x2s = {}
                for hc in range(HC):
                    x_sb = xin.tile([128, ST], BF16)
                    nc.sync.dma_start(out=x_sb, in_=xT_t[:, hc, ss])
                    if st == 0 and wgroups and wgroups[0][0] == hc:
                        g0, gn = wgroups.pop(0)
                        nc.sync.dma_start(out=wq_sb[:, g0:g0 + gn, :],
                                          in_=wq_t[:, g0:g0 + gn, :])
                        nc.sync.dma_start(out=wk_sb[:, g0:g0 + gn, :],
                                          in_=wk_t[:, g0:g0 + gn, :])
                        nc.sync.dma_start(out=wv_sb[:, g0:g0 + gn, :],
                                          in_=wv_t[:, g0:g0 + gn, :])
                    if st > 0 and hc in (6, 14, 22):
                        # o_proj weights trickle in during tiles 1-3
                        ht = 3 * (st - 1) + (hc - 6) // 8
                        if ht < NHT:
                            nc.sync.dma_start(
                                out=wo_sb[:, :, bass.ts(ht, HT)],
                                in_=wo_t[:, :, bass.ts(ht, HT)],
                            )
                    x2_sb = x2b.tile([128, ST], BF16)
                    nc.scalar.square(x2_sb, x_sb)
                    x2s[hc] = x2_sb
                    st_, sp_ = (hc == 0), (hc == HC - 1)
                    for m in range(qh):
                        nc.tensor.matmul(
                            q_ps[m], wq_sb[:, hc, bass.ts(m, 128)], x_sb,
                            start=st_, stop=sp_,
                        )
                    nc.tensor.matmul(k_ps, wk_sb[:, hc, :], x_sb,
                                     start=st_, stop=sp_)
                    nc.tensor.matmul(v_ps, wv_sb[:, hc, :], x_sb,
                                     start=st_, stop=sp_)
                    # stats lag the stream so tile starts are pure
                    # projection work
                    if hc >= 8:
                        stats(hc - 8)
                    # previous s-tile's v -> natural [j, d] transposes,
                    # placed mid-tile where PSUM banks have slack
                    if st > 0 and hc in (16, 18, 20, 22):
                        jc = (st - 1) * (ST // 128) + (hc - 16) // 2
                        vt_ps = acc_ps.tile([128, 128], F32, tag="acc")
                        nc.tensor.transpose(
                            vt_ps, vT_sb[:, bass.ts(jc, 128)], ident_sb)
                        nc.scalar.copy(vnat_sb[:, jc, :], vt_ps)
                if st == 0:
                    # rope tables: after tile-0's weights, before the first
                    # evacuation needs them
                    nc.sync.dma_start(out=cos_sb, in_=cosT[:, :])
                    nc.sync.dma_start(out=sin_sb, in_=sinTs[:, :])
                for hcl in range(HC - 8, HC):
                    stats(hcl)
                # r = 1/sqrt(mean + eps) in [s-part, 4] layout, rotated back
                # to a [1, ST] row for the table broadcast
                sq_f = statp.tile([128, 4], F32, tag="stat4f",
                                  name="sq_f")
                nc.vector.tensor_add(sq_f, sq_acc[:, 0:4], sq_acc[:, 4:8])
                sd_sb = statp.tile([128, 4], F32, tag="stat4",
                                   name="sd_sb")
                nc.scalar.activation(
                    sd_sb, sq_f, mybir.ActivationFunctionType.Sqrt,
                    bias=eps_sb, scale=1.0 / h,
                )
                # reciprocals spread to columns 0/32/64/96 so the
                # transpose lands them on 32-aligned partitions (DVE reads
                # require 32-aligned partition bases)
                rr_sb = statp.tile([128, 4, 32], F32, tag="stat4b",
                                   name="rr_sb")
                for c in range(4):
                    nc.vector.reciprocal(rr_sb[:, c, 0:1], sd_sb[:, c:c + 1])
                rT_ps = scr[:, 16:144]
                nc.tensor.transpose(rT_ps, rr_sb, ident_sb)
                rf_sb = statp.tile([1, ST], BF16, tag="statfb",
                                   name="rf_sb", bufs=3)
                for c in range(4):
                    nc.vector.tensor_copy(
                        rf_sb[0:1, bass.ts(c, 128)],
                        rT_ps[32 * c:32 * c + 1, :])
                R_t = tabp.tile([128, ST], BF16, tag="R", name="R_t")
                nc.gpsimd.partition_broadcast(R_t, rf_sb)
                cp_t = tabp.tile([128, ST], BF16, tag="cp", name="cp_t")
                nc.vector.tensor_mul(cp_t, cos_sb[:, ss], R_t)
                sp_t = tabp.tile([128, ST], BF16, tag="sp", name="sp_t")
                nc.vector.tensor_mul(sp_t, sin_sb[:, ss], R_t)

                # evacuation: fast ACT copy frees the PSUM bank, then
                # norm+RoPE happens SBUF-side on DVE (in place; the u-halves
                # read the raw values before the cos-multiply overwrites)
                # all PSUM->SBUF copies first, alternating ACT/DVE, so
                # the banks free ~2x faster for the next tile; norm+RoPE
                # then happens SBUF-side on DVE (in place; the u-halves
                # read the raw values before the cos-multiply overwrites)
                evacs = [(k_ps, kT_sb[:, ss])]
                evacs += [(q_ps[m], qT_sb[:, m, ss]) for m in range(qh)]
                for idx, (src_ps, dst) in enumerate(evacs):
                    if idx % 2 == 0:
                        nc.scalar.copy(dst, src_ps)
                    else:
                        nc.vector.tensor_copy(dst, src_ps)
                nc.scalar.copy(vT_sb[:, ss], v_ps)

                def rope_rot(dst):
                    u_sb = ropep.tile([128, ST], BF16, tag="u",
                                      name="u_sb", bufs=3)
                    nc.vector.tensor_mul(
                        u_sb[0:64, :], dst[64:128, :], sp_t[64:128, :])
                    nc.vector.tensor_mul(
                        u_sb[64:128, :], dst[0:64, :], sp_t[0:64, :])
                    nc.vector.tensor_mul(dst, dst, cp_t)
                    nc.vector.tensor_add(dst, dst, u_sb)

                for _, dst in evacs:
                    rope_rot(dst)
                nc.vector.tensor_mul(vT_sb[:, ss], vT_sb[:, ss], R_t)

            for st in range(NST):
                pass_b(st)

            # ---- phase 2: last s-tile's v transposes + the last wo
            # chunks (the rest streamed during tiles 1-3) ----
            if stop_after != "p1":
                for ht in range(min(3 * (NST - 1), NHT), NHT):
                    nc.sync.dma_start(
                        out=wo_sb[:, :, bass.ts(ht, HT)],
                        in_=wo_t[:, :, bass.ts(ht, HT)],
                    )
            for jc in range((NST - 1) * (ST // 128),
                            NJ if stop_after != "p1" else 0):
                vt_ps = acc_ps.tile([128, 128], F32, tag="acc")
                nc.tensor.transpose(vt_ps, vT_sb[:, bass.ts(jc, 128)],
                                    ident_sb)
                nc.scalar.copy(vnat_sb[:, jc, :], vt_ps)

            # attn^T reuses the wk/wv slots (heads 0-1 / 2-3)
            attnT_h = [
                persist.tile([128, 2, s], BF16, tag="wk", name="attnT01"),
                persist.tile([128, 2, s], BF16, tag="wv", name="attnT23"),
            ]

            def attn_slice(m, sl):
                return attnT_h[m // 2][:, m % 2, sl]

            # ---- phase 3 + 4 interleaved: attention per i-tile (both head
            # pairs); the previous i-tile's o_proj pieces are emitted one at
            # a time between j-chunks so they are available as PE filler
            # during the exp->mask->AV latency chains
            o_pending = []

            def o_proj_piece(sc, ht):
                scs = bass.ts(sc, 128)
                o_ps = acc_ps.tile([128, HT], F32, tag="acc")
                for m in range(qh):
                    nc.tensor.matmul(
                        o_ps, attn_slice(m, scs),
                        wo_sb[:, m, bass.ts(ht, HT)],
                        start=(m == 0), stop=(m == qh - 1),
                    )
                o_sb = outp.tile([128, HT], BF16)
                if (sc + ht) % 2 == 0:
                    nc.scalar.copy(o_sb, o_ps)
                else:
                    nc.vector.tensor_copy(o_sb, o_ps)
                nc.sync.dma_start(
                    out=out[scs, bass.ts(ht, HT)], in_=o_sb
                )

            def emit_o(n):
                for _ in range(n):
                    if o_pending:
                        o_proj_piece(*o_pending.pop(0))

            # attention i-tiles: narrow at the start (shrinks the
            # filler-less warmup) and at the end (shrinks the un-overlapped
            # o_proj tail), wide in the middle; narrower diag tiles also
            # skip more of the causal upper triangle
            ATILES = [(0, 256), (256, 256), (512, 512), (1024, 512),
                      (1536, 512)]

            def attn_tile(hp, i0, width, o_per_jc):
                NC2 = width // 128
                heads = (2 * hp, 2 * hp + 1)
                iss = slice(i0, i0 + width)
                av_ps = [acc_ps.tile([128, width], F32, tag="acc",
                                     name=f"av_ps{i}") for i in range(2)]
                njc = (i0 + width) // 128
                # small warmup tiles: Z via the direct ones-matmul into a
                # [1, w] row (cheap at small njc*w, and the row layout needs
                # no transpose/copies -- a much shorter evac chain).  Large
                # tiles use flipped single-shot matmuls + SBUF accumulation.
                rowz = njc <= 4
                if rowz:
                    zrow = [acc_ps.tile([1, width], F32, tag="acc",
                                        name=f"zrow{i}") for i in range(2)]
                else:
                    scr = acc_ps.tile([128, 320], F32, tag="acc",
                                      name="a_scr")
                    z_acc = statp.tile([128, 8 * NC2], F32, tag="zacc",
                                       name="z_acc")
                    nc.vector.memset(z_acc, 0.0)
                o_carry = 0.0
                nrm = 0      # normal-chunk counter for pair batching
                for jc in range(njc):
                    st_, sp_ = (jc == 0), (jc == njc - 1)
                    rel = jc * 128 - i0
                    diag = rel + 128 > 0
                    # diag chunks only need query columns >= the diagonal;
                    # restrict on wide tiles (bf16 scores run 1 cyc/row at
                    # any width; the f32r AV at 128 wide costs the same 4x)
                    restr = diag and rel > 0
                    if restr:
                        if not rowz and nrm % 2 == 1:
                            # fold the unpaired normal chunk's parity-0
                            # region before restricted chunks reuse it
                            nc.vector.tensor_add(z_acc[:, 0:4 * NC2],
                                                 z_acc[:, 0:4 * NC2],
                                                 scr[:, 0:4 * NC2])
                            nrm += 1
                        nskip = rel // 128
                        for i in range(2):
                            s_ps = acc_ps.tile([128, width], F32, tag="acc",
                                               name=f"s_ps{i}")
                            nc.tensor.matmul(
                                s_ps[:, rel:width],
                                kT_sb[:, bass.ts(jc, 128)],
                                qT_sb[:, heads[i], i0 + rel:i0 + width],
                                start=True, stop=True,
                            )
                            p_sb = probs.tile([128, width], BF16, tag="p",
                                              name=f"p_sb{i}", bufs=10)
                            nc.scalar.activation(
                                p_sb[:, rel:width], s_ps[:, rel:width],
                                mybir.ActivationFunctionType.Exp,
                                scale=scale,
                            )
                            nc.gpsimd.affine_select(
                                out=p_sb[:, rel:rel + 128],
                                in_=p_sb[:, rel:rel + 128],
                                pattern=[[1, 128]],
                                compare_op=mybir.AluOpType.is_ge,
                                fill=0.0, base=0, channel_multiplier=-1,
                            )
                            nc.tensor.matmul(
                                av_ps[i][:, rel:width], vnat_sb[:, jc, :],
                                p_sb[:, rel:width],
                                start=False, stop=sp_,
                                skip_group_check=True,
                            )
                            if rowz:
                                nc.tensor.matmul(
                                    zrow[i][:, rel:width], ones_bf,
                                    p_sb[:, rel:width],
                                    start=False, stop=sp_,
                                    skip_group_check=True,
                                )
                            else:
                                for c in range(nskip, NC2):
                                    zo = 2 * NC2 * i + 2 * c
                                    nc.tensor.matmul(
                                        scr[:, zo:zo + 1],
                                        p_sb[:, bass.ts(c, 128)], ones_bf,
                                        start=True, stop=True,
                                    )
                        if not rowz:
                            # per-chunk fold of the freshly-written slots
                            for i in range(2):
                                zo = 2 * NC2 * i + 2 * nskip
                                hi = 2 * NC2 * (i + 1)
                                nc.vector.tensor_add(
                                    z_acc[:, zo:hi], z_acc[:, zo:hi],
                                    scr[:, zo:hi])
                    else:
                        zoff = 4 * NC2 * (nrm % 2)
                        for i in range(2):
                            s_ps = acc_ps.tile([128, width], F32, tag="acc",
                                               name=f"s_ps{i}")
                            nc.tensor.matmul(
                                s_ps, kT_sb[:, bass.ts(jc, 128)],
                                qT_sb[:, heads[i], iss],
                                start=True, stop=True,
                            )
                            p_sb = probs.tile([128, width], BF16, tag="p",
                                              name=f"p_sb{i}", bufs=10)
                            nc.scalar.activation(
                                p_sb, s_ps,
                                mybir.ActivationFunctionType.Exp,
                                scale=scale,
                            )
                            if diag:
                                # rel == 0 here: only the first 128-col
                                # block is triangular; later columns are
                                # always kept, so mask just that block
                                nc.gpsimd.affine_select(
                                    out=p_sb[:, 0:128], in_=p_sb[:, 0:128],
                                    pattern=[[1, 128]],
                                    compare_op=mybir.AluOpType.is_ge,
                                    fill=0.0,
                                    base=0,
                                    channel_multiplier=-1,
                                )
                            nc.tensor.matmul(av_ps[i], vnat_sb[:, jc, :],
                                             p_sb, start=st_, stop=sp_,
                                             skip_group_check=True)
                            if rowz:
                                nc.tensor.matmul(
                                    zrow[i], ones_bf, p_sb,
                                    start=st_, stop=sp_,
                                    skip_group_check=True,
                                )
                            else:
                                for c in range(NC2):
                                    zo = zoff + 2 * NC2 * i + 2 * c
                                    nc.tensor.matmul(
                                        scr[:, zo:zo + 1],
                                        p_sb[:, bass.ts(c, 128)], ones_bf,
                                        start=True, stop=True,
                                    )
                        if not rowz and nrm % 2 == 1:
                            nc.vector.tensor_add(z_acc, z_acc,
                                                 scr[:, 0:8 * NC2])
                        nrm += 1
                    # diag chunks have the longest latency chains: give
                    # them double filler weight
                    o_carry += 3.0 * o_per_jc if diag else o_per_jc
                    if o_carry >= 1.0:
                        n = int(o_carry)
                        o_carry -= n
                        emit_o(n)
                if not rowz and nrm % 2 == 1:
                    # odd normal count with no restricted chunks after
                    nc.vector.tensor_add(z_acc[:, 0:4 * NC2],
                                         z_acc[:, 0:4 * NC2],
                                         scr[:, 0:4 * NC2])
                if not rowz:
                    z_f = statp.tile([128, 4 * NC2], F32, tag="zacc_f",
                                     name="z_f")
                    nc.vector.tensor_add(z_f, z_acc[:, 0:4 * NC2],
                                         z_acc[:, 4 * NC2:8 * NC2])
                for i, hh in enumerate(heads):
                    if rowz:
                        zf_sb = statp.tile([1, width], F32, tag="statf",
                                           name="zf_sb", bufs=3)
                        nc.vector.reciprocal(zf_sb, zrow[i])
                        ZR_sb = bcastp.tile([128, width], F32, tag="bcast",
                                            name="ZR_sb")
                        nc.gpsimd.partition_broadcast(ZR_sb, zf_sb)
                        nc.vector.tensor_mul(attn_slice(hh, iss), av_ps[i],
                                             ZR_sb)
                        continue
                    zr_sb = statp.tile([128, NC2, 32], F32, tag="stat4b",
                                       name="zr_sb")
                    for c in range(NC2):
                        zo = 2 * NC2 * i + 2 * c
                        nc.vector.reciprocal(
                            zr_sb[:, c, 0:1], z_f[:, zo:zo + 1])
                    zrT_ps = scr[0:32 * NC2, 64 + 128 * i:192 + 128 * i]
                    nc.tensor.transpose(zrT_ps, zr_sb, ident_sb)
                    zf_sb = statp.tile([1, width], F32, tag="statf",
                                       name="zf_sb", bufs=3)
                    for c in range(NC2):
                        nc.vector.tensor_copy(
                            zf_sb[0:1, bass.ts(c, 128)],
                            zrT_ps[32 * c:32 * c + 1, :])
                    ZR_sb = bcastp.tile([128, width], F32, tag="bcast",
                                        name="ZR_sb")
                    nc.gpsimd.partition_broadcast(ZR_sb, zf_sb)
                    nc.vector.tensor_mul(attn_slice(hh, iss), av_ps[i],
                                         ZR_sb)

            if stop_after not in ("p1", "p2"):
                for i0, width in ATILES:
                    # pieces from the previous i-tile, spread across this
                    # tile's 2 * njc j-chunk iterations
                    njc = (i0 + width) // 128
                    nc2 = width // 128
                    o_per_jc = len(o_pending) / (2.0 * (njc + 2 * nc2))
                    for hp in range(qh // 2):
                        attn_tile(hp, i0, width, o_per_jc)
                    if stop_after is None:
                        o_pending.extend(
                            (sc, ht)
                            for sc in range(i0 // 128, (i0 + width) // 128)
                            for ht in range(NHT)
                        )
                emit_o(len(o_pending))

    nc.compile()
    return nc


def make_core_inputs(hidden_states, cos, sin, norm_w, wq, wk, wv, wo,
                     s=S, h=H, qh=QH, n_cores=N_CORES):
    """Host-side sharding + layout preparation. Returns list of in_maps."""
    import ml_dtypes

    bf16 = ml_dtypes.bfloat16
    dq = qh * HD
    dkv = DKV
    x = np.asarray(hidden_states, dtype=np.float32).reshape(s, h)
    nw = np.asarray(norm_w, dtype=np.float32)
    xT = np.ascontiguousarray(x.T.astype(bf16))         # [h, s]
    cosT = np.ascontiguousarray(
        np.asarray(cos, np.float32).reshape(s, HD).T.astype(bf16))
    sinT = np.ascontiguousarray(np.asarray(sin, np.float32).reshape(s, HD).T)
    # swapped/sign-flipped sin table: rows 0:64 = +sin_half, 64:128 = -sin_half
    sin_half = sinT[0:64]
    sinTs = np.ascontiguousarray(
        np.concatenate([sinT[64:128], -sin_half], axis=0).astype(bf16))
    # fold norm_w into the projection weights
    wq_f = np.asarray(wq, np.float32) * nw[:, None]
    wk_f = np.asarray(wk, np.float32) * nw[:, None]
    wv_f = np.asarray(wv, np.float32) * nw[:, None]
    wo_f = np.asarray(wo, np.float32)

    in_maps = []
    for c in range(n_cores):
        in_maps.append({
            "xT": xT,
            "wq": np.ascontiguousarray(wq_f[:, c * dq:(c + 1) * dq].astype(bf16)),
            "wk": np.ascontiguousarray(wk_f[:, c * dkv:(c + 1) * dkv].astype(bf16)),
            "wv": np.ascontiguousarray(wv_f[:, c * dkv:(c + 1) * dkv].astype(bf16)),
            "wo": np.ascontiguousarray(wo_f[c * dq:(c + 1) * dq, :].astype(bf16)),
            "cosT": cosT,
            "sinTs": sinTs,
        })
    return in_maps


_NC_CACHE = {}


def kernel(hidden_states, cos, sin, norm_w, wq, wk, wv, wo):
    from concourse.bass_utils import run_bass_kernel_spmd

    if "nc" not in _NC_CACHE:
        _NC_CACHE["nc"] = build_bass()
    nc = _NC_CACHE["nc"]
    in_maps = make_core_inputs(hidden_states, cos, sin, norm_w, wq, wk, wv, wo)
    res = run_bass_kernel_spmd(nc, in_maps, core_ids=list(range(N_CORES)))
    partials = [m["out"] for m in res.results]
    out = np.asarray(hidden_states, np.float32).reshape(S, H).copy()
    for p in partials:
        out += np.asarray(p, dtype=np.float32)
    return out.reshape(B, S, H)


# revision 76
# speedup vs baseline: 1.4629x; 1.0002x over previous
"""Mixtral attention layer (B=1, S=2048, H=4096, NH=32, NKV=8, HD=128) on 8
Trainium2 NeuronCores, tensor-parallel over heads.

Sharding: core c owns 4 query heads + 1 KV head (column-shard of wq/wk/wv,
row-shard of wo).  Each core computes a full [S, H] partial of the o_proj
output (bf16); the host sums the 8 partials and adds the residual (the
gather of a row-parallel matmul).

Key ideas (PE matmul cost is output-free-size x cycles/row; fp32r at
free>=256 and bf16 run at 1 cycle/row):
  * Partition-dim reductions (RMSNorm sum(x^2), softmax row-sums Z) use
    FLIPPED matmuls: the data tile is the stationary operand and a ones
    vector the moving one, so each costs ~1 output column instead of the
    tile width.  Each flipped matmul is single-shot (start+stop) into a
    per-chunk-pair region of a scratch PSUM bank (concurrently-open
    accumulation groups in one PSUM bank are illegal: a start wipes the
    other open groups); a DVE add folds pairs into an SBUF accumulator.
    The [s-on-partitions] results are rotated back to [1,s] rows with a
    PE transpose (inputs spread to columns 0/32/64/96 so the transposed
    rows land on 32-aligned partition bases, which DVE reads require).
  * x stream and all weights are bf16 (walrus forbids mixing f32r with
    bf16 in one matmul, so pairs are bf16xbf16); the attention
    internals (kT/qT/vnat/probs, all accumulated in fp32 PSUM) stay
    f32r; attnT/wo (o_proj pair) and the output partials are bf16.
  * Phase 1 fuses norm stats into the projection stream (x read once):
    per H-chunk, 6 accumulating projection matmuls + ACT square + 4
    flipped stats matmuls (lagged 4 chunks so tile starts are pure
    projection work).  Weight DMAs are chunked and interleaved with
    tile-0 x chunks so the PE starts ~4us in; wo streams during tiles
    1-3 into its own SBUF slot; v is transposed to natural [j, d]
    layout mid-next-tile.  norm_w folds into the weights on the host,
    the RMSNorm 1/rms into the RoPE cos/sin tables; q^T/k^T go straight
    to persistent SBUF (no DRAM spill).
  * Attention (causal flash-style, per head-pair per 512-wide i-tile):
    scores^T = kT.T @ qT chunkwise, exp on ACT, causal mask via GPSIMD
    affine_select on diagonal blocks, unnormalized AV accumulating in
    PSUM, Z via flipped matmuls; 1/Z applied at AV evacuation.  The
    previous i-tile's o_proj pieces are emitted one at a time between
    j-chunks so the scheduler can fill the exp->mask->AV latency chains
    with o_proj matmuls.
"""

import math

import numpy as np

import concourse.bass as bass
import concourse.tile as tile
from concourse import bacc, mybir
from concourse.masks import make_identity

F32 = mybir.dt.float32
F32R = mybir.dt.float32r
BF16 = mybir.dt.bfloat16

# Full problem dims
B, S, H, NH, NKV, HD = 1, 2048, 4096, 32, 8, 128
EPS = 1e-5
N_CORES = 8
QH = NH // N_CORES          # query heads per core = 4
DQ = QH * HD                # q columns per core = 512
DKV = (NKV // N_CORES) * HD  # kv columns per core = 128


def build_bass(s=S, h=H, qh=QH, stop_after=None):
    """Build the single-core Bass module (same NEFF on all 8 cores)."""
    ST = 512 if s >= 512 else s       # s-tile width (proj + attention i-tiles)
    NST = s // ST                     # number of s-tiles
    HC = h // 128                     # H contraction chunks
    NJ = s // 128                     # j chunks (keys)
    NHT = h // 512 if h >= 512 else 1  # h tiles for o_proj output
    HT = min(512, h)
    WGRP = 4                          # h-chunks per weight DMA group
    dq = qh * HD
    scale = 1.0 / math.sqrt(HD)

    nc = bacc.Bacc(None, target_bir_lowering=False)

    xT = nc.dram_tensor("xT", [h, s], BF16, kind="ExternalInput")
    wq = nc.dram_tensor("wq", [h, dq], BF16, kind="ExternalInput")
    wk = nc.dram_tensor("wk", [h, DKV], BF16, kind="ExternalInput")
    wv = nc.dram_tensor("wv", [h, DKV], BF16, kind="ExternalInput")
    wo = nc.dram_tensor("wo", [dq, h], BF16, kind="ExternalInput")
    cosT = nc.dram_tensor("cosT", [HD, s], BF16, kind="ExternalInput")
    sinTs = nc.dram_tensor("sinTs", [HD, s], BF16, kind="ExternalInput")
    out = nc.dram_tensor("out", [s, h], BF16, kind="ExternalOutput")

    xT_t = xT.rearrange("(ho hi) s -> hi ho s", hi=128)
    wq_t = wq.rearrange("(ho hi) d -> hi ho d", hi=128)
    wk_t = wk.rearrange("(ho hi) d -> hi ho d", hi=128)
    wv_t = wv.rearrange("(ho hi) d -> hi ho d", hi=128)
    wo_t = wo.rearrange("(do di) h -> di do h", di=128)

    with tile.TileContext(nc) as tc:
        with (
            tc.tile_pool(name="persist", bufs=1) as persist,
            tc.tile_pool(name="xin", bufs=5) as xin,
            tc.tile_pool(name="x2b", bufs=9) as x2b,
            tc.tile_pool(name="rope", bufs=3) as ropep,
            tc.tile_pool(name="statp", bufs=6) as statp,
            tc.tile_pool(name="tabp", bufs=3) as tabp,
            tc.tile_pool(name="bcastp", bufs=3) as bcastp,
            tc.tile_pool(name="probs", bufs=6) as probs,
            tc.tile_pool(name="outp", bufs=6) as outp,
            tc.tile_pool(name="acc_ps", bufs=8, space="PSUM") as acc_ps,
        ):
            # ---- persistent SBUF tensors ----
            # Slot reuse chains (same tag, sequential lifetimes):
            #   wq (8MB) -> wo (8MB)         tag "bigw"
            #   wk (2MB) -> attnT heads 0-1  tag "wk"
            #   wv (2MB) -> attnT heads 2-3  tag "wv"
            #   cos (1MB) -> v natural (1MB) tag "cosvnat"
            wq_sb = persist.tile([128, HC, dq], BF16, tag="bigw")
            wk_sb = persist.tile([128, HC, DKV], BF16, tag="wk")
            wv_sb = persist.tile([128, HC, DKV], BF16, tag="wv")
            cos_sb = persist.tile([128, s], BF16, tag="cos")
            sin_sb = persist.tile([128, s], BF16, tag="sin")
            ones_f = persist.tile([128, 1], F32, tag="ones_f")
            ones_sb = persist.tile([128, 2], F32R, tag="ones")
            ones_bf = persist.tile([128, 1], BF16, tag="ones_bf")
            eps_sb = persist.tile([128, 1], F32, tag="eps")
            ident_sb = persist.tile([128, 128], F32, tag="ident")
            kT_sb = persist.tile([128, s], BF16, tag="kT")
            vT_sb = persist.tile([128, s], F32, tag="vT")
            qT_sb = persist.tile([128, qh, s], BF16, tag="qT")
            wo_sb = persist.tile([128, qh, h], BF16, tag="wo")
            vnat_sb = persist.tile([128, NJ, 128], BF16, tag="vnat")

            nc.vector.memset(ones_f, 1.0)
            nc.scalar.copy(ones_sb[:, 0:1], ones_f)
            nc.scalar.copy(ones_sb[:, 1:2], ones_f)
            nc.scalar.copy(ones_bf, ones_f)
            nc.vector.memset(eps_sb, EPS)
            make_identity(nc, ident_sb)

            # ---- phase 1: fused norm stats + q/k/v projections off a single
            # fp32r x^T stream; weight DMAs interleaved with tile-0 x chunks
            def pass_b(st):
                ss = bass.ts(st, ST)
                q_ps = [acc_ps.tile([128, ST], F32, tag="acc", name=f"q_ps{m}")
                        for m in range(qh)]
                k_ps = acc_ps.tile([128, ST], F32, tag="acc", name="k_ps")
                v_ps = acc_ps.tile([128, ST], F32, tag="acc", name="v_ps")
                # sum(x^2) accumulates in SBUF: each chunk's flipped
                # matmuls are single-shot (start+stop) into a transient PSUM
                # tile -- concurrently-open accumulation groups in one PSUM
                # bank are illegal (a start wipes the other open groups)
                # one scratch PSUM bank per s-tile: cols 0:8 hold the
                # per-chunk-pair stats groups, 16:144 the r transpose
                scr = acc_ps.tile([128, 144], F32, tag="acc", name="p1scr")
                sq_acc = statp.tile([128, 8], F32, tag="sqacc",
                                    name="sq_acc")
                nc.vector.memset(sq_acc, 0.0)
                # weight DMA groups for tile 0: small groups first so the
                # PE can start early, then 4-chunk groups
                wgroups = [(0, 1), (1, 1), (2, 2)] + [
                    (g, WGRP) for g in range(4, HC, WGRP)]

                def stats(hc):
                    # flipped stats: sum over h of x^2 lands on the s
                    # partitions; costs ~1 output column per matmul.  Each
                    # matmul is single-shot (concurrently-open accumulation
                    # groups in one PSUM bank are illegal); a chunk pair
                    # lands in scratch cols 0:8, then one DVE add folds it
                    # into the SBUF accumulator.
                    off = 4 * (hc % 2)
                    for c in range(4):
                        nc.tensor.matmul(
                            scr[:, off + c:off + c + 1],
                            x2s[hc][:, bass.ts(c, 128)],
                            ones_bf, start=True, stop=True,
                        )
                    if hc % 2 == 1:
                        nc.vector.tensor_add(sq_acc, sq_acc, scr[:, 0:8])
                    x2s[hc] = None

                x2s = {}
                for hc in range(HC):
                    x_sb = xin.tile([128, ST], BF16)
                    nc.sync.dma_start(out=x_sb, in_=xT_t[:, hc, ss])
                    if st == 0 and wgroups and wgroups[0][0] == hc:
                        g0, gn = wgroups.pop(0)
                        nc.sync.dma_start(out=wq_sb[:, g0:g0 + gn, :],
                                          in_=wq_t[:, g0:g0 + gn, :])
                        nc.sync.dma_start(out=wk_sb[:, g0:g0 + gn, :],
                                          in_=wk_t[:, g0:g0 + gn, :])
                        nc.sync.dma_start(out=wv_sb[:, g0:g0 + gn, :],
                                          in_=wv_t[:, g0:g0 + gn, :])
                    if st > 0 and hc in (6, 14, 22):
                        # o_proj weights trickle in during tiles 1-3
                        ht = 3 * (st - 1) + (hc - 6) // 8
                        if ht < NHT:
                            nc.sync.dma_start(
                                out=wo_sb[:, :, bass.ts(ht, HT)],
                                in_=wo_t[:, :, bass.ts(ht, HT)],
                            )
                    x2_sb = x2b.tile([128, ST], BF16)
                    nc.scalar.square(x2_sb, x_sb)
                    x2s[hc] = x2_sb
                    st_, sp_ = (hc == 0), (hc == HC - 1)
                    for m in range(qh):
                        nc.tensor.matmul(
                            q_ps[m], wq_sb[:, hc, bass.ts(m, 128)], x_sb,
                            start=st_, stop=sp_,
                        )
                    nc.tensor.matmul(k_ps, wk_sb[:, hc, :], x_sb,
                                     start=st_, stop=sp_)
                    nc.tensor.matmul(v_ps, wv_sb[:, hc, :], x_sb,
                                     start=st_, stop=sp_)
                    # stats lag the stream so tile starts are pure
                    # projection work
                    if hc >= 8:
                        stats(hc - 8)
                    # previous s-tile's v -> natural [j, d] transposes,
                    # placed mid-tile where PSUM banks have slack
                    if st > 0 and hc in (16, 18, 20, 22):
                        jc = (st - 1) * (ST // 128) + (hc - 16) // 2
                        vt_ps = acc_ps.tile([128, 128], F32, tag="acc")
                        nc.tensor.transpose(
                            vt_ps, vT_sb[:, bass.ts(jc, 128)], ident_sb)
                        nc.scalar.copy(vnat_sb[:, jc, :], vt_ps)
                if st == 0:
                    # rope tables: after tile-0's weights, before the first
                    # evacuation needs them
                    nc.sync.dma_start(out=cos_sb, in_=cosT[:, :])
                    nc.sync.dma_start(out=sin_sb, in_=sinTs[:, :])
                for hcl in range(HC - 8, HC):
                    stats(hcl)
                # r = 1/sqrt(mean + eps) in [s-part, 4] layout, rotated back
                # to a [1, ST] row for the table broadcast
                sq_f = statp.tile([128, 4], F32, tag="stat4f",
                                  name="sq_f")
                nc.vector.tensor_add(sq_f, sq_acc[:, 0:4], sq_acc[:, 4:8])
                sd_sb = statp.tile([128, 4], F32, tag="stat4",
                                   name="sd_sb")
                nc.scalar.activation(
                    sd_sb, sq_f, mybir.ActivationFunctionType.Sqrt,
                    bias=eps_sb, scale=1.0 / h,
                )
                # reciprocals spread to columns 0/32/64/96 so the
                # transpose lands them on 32-aligned partitions (DVE reads
                # require 32-aligned partition bases)
                rr_sb = statp.tile([128, 4, 32], F32, tag="stat4b",
                                   name="rr_sb")
                for c in range(4):
                    nc.vector.reciprocal(rr_sb[:, c, 0:1], sd_sb[:, c:c + 1])
                rT_ps = scr[:, 16:144]
                nc.tensor.transpose(rT_ps, rr_sb, ident_sb)
                rf_sb = statp.tile([1, ST], BF16, tag="statfb",
                                   name="rf_sb", bufs=3)
                for c in range(4):
                    nc.vector.tensor_copy(
                        rf_sb[0:1, bass.ts(c, 128)],
                        rT_ps[32 * c:32 * c + 1, :])
                R_t = tabp.tile([128, ST], BF16, tag="R", name="R_t")
                nc.gpsimd.partition_broadcast(R_t, rf_sb)
                cp_t = tabp.tile([128, ST], BF16, tag="cp", name="cp_t")
                nc.vector.tensor_mul(cp_t, cos_sb[:, ss], R_t)
                sp_t = tabp.tile([128, ST], BF16, tag="sp", name="sp_t")
                nc.vector.tensor_mul(sp_t, sin_sb[:, ss], R_t)

                # evacuation: fast ACT copy frees the PSUM bank, then
                # norm+RoPE happens SBUF-side on DVE (in place; the u-halves
                # read the raw values before the cos-multiply overwrites)
                # all PSUM->SBUF copies first, alternating ACT/DVE, so
                # the banks free ~2x faster for the next tile; norm+RoPE
                # then happens SBUF-side on DVE (in place; the u-halves
                # read the raw values before the cos-multiply overwrites)
                evacs = [(k_ps, kT_sb[:, ss])]
                evacs += [(q_ps[m], qT_sb[:, m, ss]) for m in range(qh)]
                for idx, (src_ps, dst) in enumerate(evacs):
                    if idx % 2 == 0:
                        nc.scalar.copy(dst, src_ps)
                    else:
                        nc.vector.tensor_copy(dst, src_ps)
                nc.scalar.copy(vT_sb[:, ss], v_ps)

                def rope_rot(dst):
                    u_sb = ropep.tile([128, ST], BF16, tag="u",
                                      name="u_sb", bufs=3)
                    nc.vector.tensor_mul(
                        u_sb[0:64, :], dst[64:128, :], sp_t[64:128, :])
                    nc.vector.tensor_mul(
                        u_sb[64:128, :], dst[0:64, :], sp_t[0:64, :])
                    nc.vector.tensor_mul(dst, dst, cp_t)
                    nc.vector.tensor_add(dst, dst, u_sb)

                for _, dst in evacs:
                    rope_rot(dst)
                nc.vector.tensor_mul(vT_sb[:, ss], vT_sb[:, ss], R_t)

            for st in range(NST):
                pass_b(st)

            # ---- phase 2: last s-tile's v transposes + the last wo
            # chunks (the rest streamed during tiles 1-3) ----
            if stop_after != "p1":
                for ht in range(min(3 * (NST - 1), NHT), NHT):
                    nc.sync.dma_start(
                        out=wo_sb[:, :, bass.ts(ht, HT)],
                        in_=wo_t[:, :, bass.ts(ht, HT)],
                    )
            for jc in range((NST - 1) * (ST // 128),
                            NJ if stop_after != "p1" else 0):
                vt_ps = acc_ps.tile([128, 128], F32, tag="acc")
                nc.tensor.transpose(vt_ps, vT_sb[:, bass.ts(jc, 128)],
                                    ident_sb)
                nc.scalar.copy(vnat_sb[:, jc, :], vt_ps)

            # attn^T reuses the wk/wv slots (heads 0-1 / 2-3)
            attnT_h = [
                persist.tile([128, 2, s], BF16, tag="wk", name="attnT01"),
                persist.tile([128, 2, s], BF16, tag="wv", name="attnT23"),
            ]

            def attn_slice(m, sl):
                return attnT_h[m // 2][:, m % 2, sl]

            # ---- phase 3 + 4 interleaved: attention per i-tile (both head
            # pairs); the previous i-tile's o_proj pieces are emitted one at
            # a time between j-chunks so they are available as PE filler
            # during the exp->mask->AV latency chains
            o_pending = []

            def o_proj_piece(sc, ht):
                scs = bass.ts(sc, 128)
                o_ps = acc_ps.tile([128, HT], F32, tag="acc")
                for m in range(qh):
                    nc.tensor.matmul(
                        o_ps, attn_slice(m, scs),
                        wo_sb[:, m, bass.ts(ht, HT)],
                        start=(m == 0), stop=(m == qh - 1),
                    )
                o_sb = outp.tile([128, HT], BF16)
                if (sc + ht) % 2 == 0:
                    nc.scalar.copy(o_sb, o_ps)
                else:
                    nc.vector.tensor_copy(o_sb, o_ps)
                nc.sync.dma_start(
                    out=out[scs, bass.ts(ht, HT)], in_=o_sb
                )

            def emit_o(n):
                for _ in range(n):
                    if o_pending:
                        o_proj_piece(*o_pending.pop(0))

            # attention i-tiles: narrow at the start (shrinks the
            # filler-less warmup) and at the end (shrinks the un-overlapped
            # o_proj tail), wide in the middle; narrower diag tiles also
            # skip more of the causal upper triangle
            ATILES = [(0, 256), (256, 256), (512, 512), (1024, 512),
                      (1536, 512)]

            def attn_tile(hp, i0, width, o_per_jc):
                NC2 = width // 128
                heads = (2 * hp, 2 * hp + 1)
                iss = slice(i0, i0 + width)
                av_ps = [acc_ps.tile([128, width], F32, tag="acc",
                                     name=f"av_ps{i}") for i in range(2)]
                njc = (i0 + width) // 128
                # small warmup tiles: Z via the direct ones-matmul into a
                # [1, w] row (cheap at small njc*w, and the row layout needs
                # no transpose/copies -- a much shorter evac chain).  Large
                # tiles use flipped single-shot matmuls + SBUF accumulation.
                rowz = njc <= 4
                if rowz:
                    zrow = [acc_ps.tile([1, width], F32, tag="acc",
                                        name=f"zrow{i}") for i in range(2)]
                else:
                    scr = acc_ps.tile([128, 320], F32, tag="acc",
                                      name="a_scr")
                    z_acc = statp.tile([128, 8 * NC2], F32, tag="zacc",
                                       name="z_acc")
                    nc.vector.memset(z_acc, 0.0)
                o_carry = 0.0
                nrm = 0      # normal-chunk counter for pair batching
                for jc in range(njc):
                    st_, sp_ = (jc == 0), (jc == njc - 1)
                    rel = jc * 128 - i0
                    diag = rel + 128 > 0
                    # diag chunks only need query columns >= the diagonal;
                    # restrict on wide tiles (bf16 scores run 1 cyc/row at
                    # any width; the f32r AV at 128 wide costs the same 4x)
                    restr = diag and rel > 0
                    if restr:
                        if not rowz and nrm % 2 == 1:
                            # fold the unpaired normal chunk's parity-0
                            # region before restricted chunks reuse it
                            nc.vector.tensor_add(z_acc[:, 0:4 * NC2],
                                                 z_acc[:, 0:4 * NC2],
                                                 scr[:, 0:4 * NC2])
                            nrm += 1
                        nskip = rel // 128
                        for i in range(2):
                            s_ps = acc_ps.tile([128, width], F32, tag="acc",
                                               name=f"s_ps{i}")
                            nc.tensor.matmul(
                                s_ps[:, rel:width],
                                kT_sb[:, bass.ts(jc, 128)],
                                qT_sb[:, heads[i], i0 + rel:i0 + width],
                                start=True, stop=True,
                            )
                            p_sb = probs.tile([128, width], BF16, tag="p",
                                              name=f"p_sb{i}", bufs=10)
                            nc.scalar.activation(
                                p_sb[:, rel:width], s_ps[:, rel:width],
                                mybir.ActivationFunctionType.Exp,
                                scale=scale,
                            )
                            nc.gpsimd.affine_select(
                                out=p_sb[:, rel:rel + 128],
                                in_=p_sb[:, rel:rel + 128],
                                pattern=[[1, 128]],
                                compare_op=mybir.AluOpType.is_ge,
                                fill=0.0, base=0, channel_multiplier=-1,
                            )
                            nc.tensor.matmul(
                                av_ps[i][:, rel:width], vnat_sb[:, jc, :],
                                p_sb[:, rel:width],
                                start=False, stop=sp_,
                                skip_group_check=True,
                            )
                            if rowz:
                                nc.tensor.matmul(
                                    zrow[i][:, rel:width], ones_bf,
                                    p_sb[:, rel:width],
                                    start=False, stop=sp_,
                                    skip_group_check=True,
                                )
                            else:
                                for c in range(nskip, NC2):
                                    zo = 2 * NC2 * i + 2 * c
                                    nc.tensor.matmul(
                                        scr[:, zo:zo + 1],
                                        p_sb[:, bass.ts(c, 128)], ones_bf,
                                        start=True, stop=True,
                                    )
                        if not rowz:
                            # per-chunk fold of the freshly-written slots
                            for i in range(2):
                                zo = 2 * NC2 * i + 2 * nskip
                                hi = 2 * NC2 * (i + 1)
                                nc.vector.tensor_add(
                                    z_acc[:, zo:hi], z_acc[:, zo:hi],
                                    scr[:, zo:hi])
                    else:
                        zoff = 4 * NC2 * (nrm % 2)
                        for i in range(2):
                            s_ps = acc_ps.tile([128, width], F32, tag="acc",
                                               name=f"s_ps{i}")
                            nc.tensor.matmul(
                                s_ps, kT_sb[:, bass.ts(jc, 128)],
                                qT_sb[:, heads[i], iss],
                                start=True, stop=True,
                            )
                            p_sb = probs.tile([128, width], BF16, tag="p",
                                              name=f"p_sb{i}", bufs=10)
                            nc.scalar.activation(
                                p_sb, s_ps,
                                mybir.ActivationFunctionType.Exp,
                                scale=scale,
                            )
                            if diag:
                                # rel == 0 here: only the first 128-col
                                # block is triangular; later columns are
                                # always kept, so mask just that block
                                nc.gpsimd.affine_select(
                                    out=p_sb[:, 0:128], in_=p_sb[:, 0:128],
                                    pattern=[[1, 128]],
                                    compare_op=mybir.AluOpType.is_ge,
                                    fill=0.0,
                                    base=0,
                                    channel_multiplier=-1,
                                )
                            nc.tensor.matmul(av_ps[i], vnat_sb[:, jc, :],
                                             p_sb, start=st_, stop=sp_,
                                             skip_group_check=True)
                            if rowz:
                                nc.tensor.matmul(
                                    zrow[i], ones_bf, p_sb,
                                    start=st_, stop=sp_,
                                    skip_group_check=True,
                                )
                            else:
                                for c in range(NC2):
                                    zo = zoff + 2 * NC2 * i + 2 * c
                                    nc.tensor.matmul(
                                        scr[:, zo:zo + 1],
                                        p_sb[:, bass.ts(c, 128)], ones_bf,
                                        start=True, stop=True,
                                    )
                        if not rowz and nrm % 2 == 1:
                            nc.vector.tensor_add(z_acc, z_acc,
                                                 scr[:, 0:8 * NC2])
                        nrm += 1
                    # diag chunks have the longest latency chains: give
                    # them double filler weight
                    o_carry += 4.0 * o_per_jc if diag else o_per_jc
                    if o_carry >= 1.0:
                        n = int(o_carry)
                        o_carry -= n
                        emit_o(n)
                if not rowz and nrm % 2 == 1:
                    # odd normal count with no restricted chunks after
                    nc.vector.tensor_add(z_acc[:, 0:4 * NC2],
                                         z_acc[:, 0:4 * NC2],
                                         scr[:, 0:4 * NC2])
                if not rowz:
                    z_f = statp.tile([128, 4 * NC2], F32, tag="zacc_f",
                                     name="z_f")
                    nc.vector.tensor_add(z_f, z_acc[:, 0:4 * NC2],
                                         z_acc[:, 4 * NC2:8 * NC2])
                for i, hh in enumerate(heads):
                    if rowz:
                        zf_sb = statp.tile([1, width], F32, tag="statf",
                                           name="zf_sb", bufs=3)
                        nc.vector.reciprocal(zf_sb, zrow[i])
                        ZR_sb = bcastp.tile([128, width], F32, tag="bcast",
                                            name="ZR_sb")
                        nc.gpsimd.partition_broadcast(ZR_sb, zf_sb)
                        nc.vector.tensor_mul(attn_slice(hh, iss), av_ps[i],
                                             ZR_sb)
                        continue
                    zr_sb = statp.tile([128, NC2, 32], F32, tag="stat4b",
                                       name="zr_sb")
                    for c in range(NC2):
                        zo = 2 * NC2 * i + 2 * c
                        nc.vector.reciprocal(
                            zr_sb[:, c, 0:1], z_f[:, zo:zo + 1])
                    zrT_ps = scr[0:32 * NC2, 64 + 128 * i:192 + 128 * i]
                    nc.tensor.transpose(zrT_ps, zr_sb, ident_sb)
                    zf_sb = statp.tile([1, width], F32, tag="statf",
                                       name="zf_sb", bufs=3)
                    for c in range(NC2):
                        nc.vector.tensor_copy(
                            zf_sb[0:1, bass.ts(c, 128)],
                            zrT_ps[32 * c:32 * c + 1, :])
                    ZR_sb = bcastp.tile([128, width], F32, tag="bcast",
                                        name="ZR_sb")
                    nc.gpsimd.partition_broadcast(ZR_sb, zf_sb)
                    nc.vector.tensor_mul(attn_slice(hh, iss), av_ps[i],
                                         ZR_sb)

            if stop_after not in ("p1", "p2"):
                for i0, width in ATILES:
                    # pieces from the previous i-tile, spread across this
                    # tile's 2 * njc j-chunk iterations
                    njc = (i0 + width) // 128
                    nc2 = width // 128
                    o_per_jc = len(o_pending) / (2.0 * (njc + 3 * nc2))
                    for hp in range(qh // 2):
                        attn_tile(hp, i0, width, o_per_jc)
                    if stop_after is None:
                        o_pending.extend(
                            (sc, ht)
                            for sc in range(i0 // 128, (i0 + width) // 128)
                            for ht in range(NHT)
                        )
                emit_o(len(o_pending))

    nc.compile()
    return nc


def make_core_inputs(hidden_states, cos, sin, norm_w, wq, wk, wv, wo,
                     s=S, h=H, qh=QH, n_cores=N_CORES):
    """Host-side sharding + layout preparation. Returns list of in_maps."""
    import ml_dtypes

    bf16 = ml_dtypes.bfloat16
    dq = qh * HD
    dkv = DKV
    x = np.asarray(hidden_states, dtype=np.float32).reshape(s, h)
    nw = np.asarray(norm_w, dtype=np.float32)
    xT = np.ascontiguousarray(x.T.astype(bf16))         # [h, s]
    cosT = np.ascontiguousarray(
        np.asarray(cos, np.float32).reshape(s, HD).T.astype(bf16))
    sinT = np.ascontiguousarray(np.asarray(sin, np.float32).reshape(s, HD).T)
    # swapped/sign-flipped sin table: rows 0:64 = +sin_half, 64:128 = -sin_half
    sin_half = sinT[0:64]
    sinTs = np.ascontiguousarray(
        np.concatenate([sinT[64:128], -sin_half], axis=0).astype(bf16))
    # fold norm_w into the projection weights
    wq_f = np.asarray(wq, np.float32) * nw[:, None]
    wk_f = np.asarray(wk, np.float32) * nw[:, None]
    wv_f = np.asarray(wv, np.float32) * nw[:, None]
    wo_f = np.asarray(wo, np.float32)

    in_maps = []
    for c in range(n_cores):
        in_maps.append({
            "xT": xT,
            "wq": np.ascontiguousarray(wq_f[:, c * dq:(c + 1) * dq].astype(bf16)),
            "wk": np.ascontiguousarray(wk_f[:, c * dkv:(c + 1) * dkv].astype(bf16)),
            "wv": np.ascontiguousarray(wv_f[:, c * dkv:(c + 1) * dkv].astype(bf16)),
            "wo": np.ascontiguousarray(wo_f[c * dq:(c + 1) * dq, :].astype(bf16)),
            "cosT": cosT,
            "sinTs": sinTs,
        })
    return in_maps


_NC_CACHE = {}


def kernel(hidden_states, cos, sin, norm_w, wq, wk, wv, wo):
    from concourse.bass_utils import run_bass_kernel_spmd

    if "nc" not in _NC_CACHE:
        _NC_CACHE["nc"] = build_bass()
    nc = _NC_CACHE["nc"]
    in_maps = make_core_inputs(hidden_states, cos, sin, norm_w, wq, wk, wv, wo)
    res = run_bass_kernel_spmd(nc, in_maps, core_ids=list(range(N_CORES)))
    partials = [m["out"] for m in res.results]
    out = np.asarray(hidden_states, np.float32).reshape(S, H).copy()
    for p in partials:
        out += np.asarray(p, dtype=np.float32)
    return out.reshape(B, S, H)
